# revision 2
# baseline (speedup 1.0000x reference)
"""MAAC critic kernel v2 for Trainium2 — attention on the TensorEngine.

Data-parallel over batch on 8 cores. Per core (b_local=2048), processed in
superchunks of SC=512 (b-tiles of 128, chunks of 16):

  encoder/kvs: feature-major weights-stationary matmuls (bf16).
  stage-1 logits: per (chunk16, head) ONE PE matmul
      out[(b2*8+j), (b1*8+i)] = sum_d keys[d,(b2,j)] * sels[d,(b1,i)]
    — the b2==b1 diagonal blocks are the real logits; the rest is cheap waste.
  softmax stays in that domain: exp on the whole block (ScalarE), then one
    DVE multiply with a constant mask [(b2 j),(b1 i)] = (b2==b1)&(i!=j)
    which zeroes both the off-diagonal garbage and the self-agent term.
  stage-2: out[(b1 i), (d|Z)] = Pm^T @ [V2 | ones] — block-diag structure of
    Pm makes this exact; the appended ones-column yields the softmax
    denominator Z per (b, i) for free.  V2 = vals in [(b0*8+j), ed] layout
    via a dense DRAM round trip (the only partition reshuffle).
  normalize: DVE reciprocal of the Z columns + one broadcast multiply.
  transpose attended values back feature-major via PE transposes, then
  per-agent critic MLP, PE-transpose of q, fp32 one-hot gather with a
  first-index tie-break matching jnp.argmax.
"""
import os
import numpy as np
import ml_dtypes

import concourse.bass as bass
import concourse.tile as tile
import concourse.mybir as mybir
from concourse import bacc
from concourse import bass_utils
from concourse.masks import make_identity

F32 = mybir.dt.float32
BF16 = mybir.dt.bfloat16
bfloat16 = ml_dtypes.bfloat16

A = 8
B = 16384
S = 456
NACT = 16
IDIM = S + NACT
KPAD = 512
H = 256
E = 4
D = 64
ED = E * D               # 256
N_CORES = 8
B_LOCAL = B // N_CORES   # 2048
ALPHA = 0.01


def build_bass(b_local=B_LOCAL, lrelu_act=True, stages=5):
    """stages: 1=enc/kvs, 2=+vals/V2, 3=+s1/softmax, 4=+s2/norm/transpose, 5=all."""
    nc = bacc.Bacc("TRN2", target_bir_lowering=False, debug=False)
    SC = min(512, b_local)       # superchunk width (free dim of big matmuls)
    n_sc = b_local // SC
    NBT = SC // 128              # b-tiles per superchunk
    LRELU = mybir.ActivationFunctionType.Lrelu
    EXP = mybir.ActivationFunctionType.Exp
    IDENT = mybir.ActivationFunctionType.Identity
    MULT = mybir.AluOpType.mult
    ADD = mybir.AluOpType.add
    ISGE = mybir.AluOpType.is_ge
    MAX = mybir.AluOpType.max
    X = mybir.AxisListType.X

    # ---- DRAM I/O ----
    saT_d = nc.dram_tensor("saT", [A, b_local // min(512, b_local), 128, 4,
                                   min(512, b_local)], BF16, kind="ExternalInput")
    acts_d = nc.dram_tensor("acts", [b_local, A, NACT], F32, kind="ExternalInput")
    w_enc_d = nc.dram_tensor("w_enc", [A, KPAD, 2 * H], BF16, kind="ExternalInput")
    b_enc_d = nc.dram_tensor("b_enc", [A, 4, 128, 1], F32, kind="ExternalInput")
    w_ks_d = nc.dram_tensor("w_ks", [H, 2 * H], BF16, kind="ExternalInput")
    w_val_d = nc.dram_tensor("w_val", [H, ED], BF16, kind="ExternalInput")
    bv_d = nc.dram_tensor("bv", [1, 2 * ED], BF16, kind="ExternalInput")
    w_c1_d = nc.dram_tensor("w_c1", [A, 2 * H, H], BF16, kind="ExternalInput")
    b_c1_d = nc.dram_tensor("b_c1", [A, 2, 128, 1], F32, kind="ExternalInput")
    w_c2_d = nc.dram_tensor("w_c2", [A, H, NACT], BF16, kind="ExternalInput")
    b_c2_d = nc.dram_tensor("b_c2", [A, NACT, 1], F32, kind="ExternalInput")
    dmask_d = nc.dram_tensor("dmask", [128, 128], BF16, kind="ExternalInput")
    w16_d = nc.dram_tensor("w16", [128, A * NACT], F32, kind="ExternalInput")
    q_d = nc.dram_tensor("q", [b_local, A], F32, kind="ExternalOutput")

    with tile.TileContext(nc) as tc:
        import contextlib
        with contextlib.ExitStack() as ctx:
            wp = ctx.enter_context(tc.tile_pool(name="wp", bufs=1))
            xp = ctx.enter_context(tc.tile_pool(name="xp", bufs=2))
            scp = ctx.enter_context(tc.tile_pool(name="scp", bufs=1))
            btp = ctx.enter_context(tc.tile_pool(name="btp", bufs=2))
            pup = ctx.enter_context(tc.tile_pool(name="pup", bufs=2))
            hp = ctx.enter_context(tc.tile_pool(name="hp", bufs=2))
            pmm = ctx.enter_context(tc.tile_pool(name="pmm", bufs=2, space="PSUM"))
            ps1 = ctx.enter_context(tc.tile_pool(name="ps1", bufs=1, space="PSUM"))
            ps2 = ctx.enter_context(tc.tile_pool(name="ps2", bufs=1, space="PSUM"))
            ptp = ctx.enter_context(tc.tile_pool(name="ptp", bufs=2, space="PSUM"))
            dp = ctx.enter_context(tc.tile_pool(name="dp", bufs=2, space="DRAM"))

            # ---- resident weights ----
            w_enc_sb, b_enc_sb = [], []
            w_c1_sb, b_c1_sb, w_c2_sb, b_c2_sb = [], [], [], []
            for a in range(A):
                w = wp.tile([128, 4, 2 * H], BF16, name=f"w_enc{a}")
                nc.sync.dma_start(out=w, in_=w_enc_d[a].rearrange("(kt p) m -> p kt m", p=128))
                w_enc_sb.append(w)
                bt_ = wp.tile([128, 4, 1], F32, name=f"b_enc{a}")
                nc.sync.dma_start(out=bt_, in_=b_enc_d[a].rearrange("kt p one -> p kt one"))
                b_enc_sb.append(bt_)
                w1 = wp.tile([128, 4, H], BF16, name=f"w_c1{a}")
                nc.sync.dma_start(out=w1, in_=w_c1_d[a].rearrange("(kt p) m -> p kt m", p=128))
                w_c1_sb.append(w1)
                b1 = wp.tile([128, 2, 1], F32, name=f"b_c1{a}")
                nc.sync.dma_start(out=b1, in_=b_c1_d[a].rearrange("kt p one -> p kt one"))
                b_c1_sb.append(b1)
                w2 = wp.tile([128, 2, NACT], BF16, name=f"w_c2{a}")
                nc.sync.dma_start(out=w2, in_=w_c2_d[a].rearrange("(kt p) m -> p kt m", p=128))
                w_c2_sb.append(w2)
                b2 = wp.tile([NACT, 1], F32, name=f"b_c2{a}")
                nc.sync.dma_start(out=b2, in_=b_c2_d[a])
                b_c2_sb.append(b2)
            w_ks_sb = wp.tile([128, 2, 2 * H], BF16, name="w_ks")
            nc.sync.dma_start(out=w_ks_sb, in_=w_ks_d.rearrange("(kt p) m -> p kt m", p=128))
            w_val_sb = wp.tile([128, 2, ED], BF16, name="w_val")
            nc.sync.dma_start(out=w_val_sb, in_=w_val_d.rearrange("(kt p) m -> p kt m", p=128))
            bv2_sb = wp.tile([1, 2 * ED], BF16, name="bv2")
            nc.sync.dma_start(out=bv2_sb, in_=bv_d[:, :])
            ones_sb = wp.tile([1, 128], BF16, name="ones")
            nc.vector.memset(ones_sb, 1.0)
            dmask_sb = wp.tile([128, 128], BF16, name="dmask")
            nc.sync.dma_start(out=dmask_sb, in_=dmask_d[:, :])
            w16_sb = wp.tile([128, A * NACT], F32, name="w16")
            nc.sync.dma_start(out=w16_sb, in_=w16_d[:, :])
            identb = wp.tile([128, 128], BF16, name="identb")
            make_identity(nc, identb)
            ident16 = wp.tile([16, 16], F32, name="ident16")
            make_identity(nc, ident16)

            RELU = mybir.ActivationFunctionType.Relu
            b2_cache = {}

            def evict_lrelu(psum_ap, out_ap, bias):
                """out = lrelu(psum + bias); bias is a [128,1] AP or None."""
                if lrelu_act:
                    nc.scalar.activation(out=out_ap, in_=psum_ap, func=LRELU,
                                         bias=bias if bias is not None else 0.0,
                                         scale=1.0, alpha=ALPHA)
                    return
                # CoreSim fallback: lrelu(y) = y + relu(-(1-a)*y), y = x + b
                n = psum_ap.shape[-1]
                r = xp.tile([128, SC], F32, tag="lrtmp", name="lrtmp")
                rr = r[:, :n] if n != SC else r
                bias2 = 0.0
                if bias is not None:
                    key = (bias.tensor.name, bias.offset)
                    if key not in b2_cache:
                        b2 = wp.tile([128, 1], F32, name=f"b2_{len(b2_cache)}")
                        nc.vector.tensor_scalar_mul(out=b2, in0=bias,
                                                    scalar1=-(1.0 - ALPHA))
                        b2_cache[key] = b2
                    bias2 = b2_cache[key]
                nc.scalar.activation(out=rr, in_=psum_ap, func=RELU,
                                     bias=bias2, scale=-(1.0 - ALPHA))
                nc.vector.scalar_tensor_tensor(
                    out=out_ap, in0=psum_ap,
                    scalar=bias if bias is not None else 0.0,
                    op0=ADD, in1=rr, op1=ADD)

            for sc in range(n_sc):
                sc0 = sc * SC
                sa_enc = scp.tile([128, A, 2, SC], BF16, tag="sa_enc", name="sa_enc")
                s_enc = scp.tile([128, A, 2, SC], BF16, tag="s_enc", name="s_enc",
                                 bufs=2)
                # keys/sels stored feature-major with cols (chunk, b2, j) so the
                # s1 matmul operands are contiguous 128-col slices
                keys_fm = [scp.tile([128, SC * A], BF16, tag=f"keys{t}", name=f"keys{t}") for t in range(2)]
                sels_fm = [scp.tile([128, SC * A], BF16, tag=f"sels{t}", name=f"sels{t}") for t in range(2)]
                otherT = [scp.tile([128, A * SC], BF16, tag=f"oT{t}", name=f"oT{t}") for t in range(2)]
                allq_bm = scp.tile([128, NBT * 128], F32, tag="allq_bm", name="allq_bm")

                # ---------- encoder + keys/sels (feature-major) ----------
                for a in range(A):
                    x = xp.tile([128, 4, SC], BF16, tag="x", name="x")
                    nc.sync.dma_start(out=x, in_=saT_d[a, sc])
                    for mt in range(4):
                        ps = pmm.tile([128, SC], F32, tag="mm", name="mm")
                        for kt in range(4):
                            nc.tensor.matmul(out=ps,
                                             lhsT=w_enc_sb[a][:, kt, mt * 128:(mt + 1) * 128],
                                             rhs=x[:, kt, :],
                                             start=(kt == 0), stop=(kt == 3))
                        dst = sa_enc[:, a, mt, :] if mt < 2 else s_enc[:, a, mt - 2, :]
                        evict_lrelu(ps, dst, b_enc_sb[a][:, mt, :])
                    # keys (from sa_enc) and sels (from s_enc), feature-major,
                    # evicted into (chunk, j=a, b2) interleaved column order
                    # (16-wide contiguous runs; attention row rank = j*16+b2)
                    def ks_dst(t, a):
                        return bass.AP(tensor=t.tensor, offset=t.offset + a * 16,
                                       ap=[t.ap[0], [128, SC // 16], [1, 16]])
                    for et in range(2):
                        ps = pmm.tile([128, SC], F32, tag="mm", name="mm")
                        for kt in range(2):
                            nc.tensor.matmul(out=ps,
                                             lhsT=w_ks_sb[:, kt, et * 128:(et + 1) * 128],
                                             rhs=sa_enc[:, a, kt, :],
                                             start=(kt == 0), stop=(kt == 1))
                        nc.vector.tensor_copy(out=ks_dst(keys_fm[et], a), in_=ps)
                        ps = pmm.tile([128, SC], F32, tag="mm", name="mm")
                        for kt in range(2):
                            nc.tensor.matmul(out=ps,
                                             lhsT=w_ks_sb[:, kt, H + et * 128:H + (et + 1) * 128],
                                             rhs=s_enc[:, a, kt, :],
                                             start=(kt == 0), stop=(kt == 1))
                        nc.vector.tensor_copy(out=ks_dst(sels_fm[et], a), in_=ps)

                # ---------- per b-tile: vals, V2 bounce, s1, softmax, s2 ----------
                for bt in range(NBT if stages >= 2 else 0):
                    bt0 = bt * 128
                    # vals b-major [128b, (j, ed)]
                    vals_bm = btp.tile([128, A, ED], BF16, tag="vb", name="vb", bufs=1)
                    for a in range(A):
                        ps = pmm.tile([128, SC], F32, tag="mm", name="mm")
                        for kt in range(2):
                            nc.tensor.matmul(out=ps[:, 0:ED],
                                             lhsT=sa_enc[:, a, kt, bt0:bt0 + 128],
                                             rhs=w_val_sb[:, kt, :],
                                             start=(kt == 0), stop=False)
                        nc.tensor.matmul(out=ps[:, 0:ED], lhsT=ones_sb,
                                         rhs=bv2_sb[:, 0:ED], start=False, stop=True)
                        evict_lrelu(ps[:, 0:ED], vals_bm[:, a, :], None)
                    # V2 via DRAM bounce: vdram [c][j][b0][ed]; row rank = j*16+b0
                    vdram = dp.tile([8, A, 16, ED], BF16, tag="vd", name="vd")
                    for j in range(A):
                        nc.sync.dma_start(
                            out=bass.AP(tensor=vdram.tensor, offset=vdram.offset + j * 16 * ED,
                                        ap=[[A * 16 * ED, 8], [ED, 16], [1, ED]]),
                            in_=vals_bm[:, j, :])
                    V2C = 4 * 65  # 260 cols per chunk: 4 x (64 vals + ones col)
                    v2 = btp.tile([128, 8, V2C], BF16, tag="v2", name="v2")
                    for c in range(8):
                        src = bass.AP(tensor=vdram.tensor, offset=vdram.offset + c * A * 16 * ED,
                                      ap=[[ED, 128], [D, 4], [1, D]])
                        dst = bass.AP(tensor=v2.tensor, offset=v2.offset + c * V2C,
                                      ap=[v2.ap[0], [65, 4], [1, D]])
                        nc.scalar.dma_start(out=dst, in_=src)
                    # ones columns at e*65+64
                    nc.vector.memset(
                        bass.AP(tensor=v2.tensor, offset=v2.offset + D,
                                ap=[v2.ap[0], [V2C, 8], [65, 4]]), 1.0)

                    if stages < 3:
                        continue
                    othBM = btp.tile([128, 8, 4 * 65], BF16, tag="ob", name="ob")
                    for c in range(8):
                        # s1: logits c2-block per head.  Base-64 operands
                        # (odd heads) must target a psum tile at bank offset 0,
                        # so e1/e3 get their own tiles.
                        cg = bt * 8 + c  # chunk index within superchunk
                        pev = ps1.tile([128, 256], F32, tag="s1e", name="s1e")
                        pod = [ps1.tile([128, 128], F32, tag=f"s1o{k}", name=f"s1o{k}")
                               for k in range(2)]
                        for e in range(4):
                            et, eh = e // 2, (e % 2) * 64
                            lhsT = _ap64(keys_fm[et], eh, cg * 128, SC)
                            rhs = _ap64(sels_fm[et], eh, cg * 128, SC)
                            out = (pev[:, (e // 2) * 128:(e // 2 + 1) * 128]
                                   if e % 2 == 0 else pod[e // 2])
                            nc.tensor.matmul(out=out, lhsT=lhsT, rhs=rhs,
                                             start=True, stop=True)
                        # exp (scale 1/sqrt(D)); pu cols stay in (e, b1, i) order
                        pu = pup.tile([128, 512], BF16, tag="pu", name="pu")
                        escale = 1.0 / np.sqrt(np.float32(D))
                        nc.scalar.activation(
                            out=bass.AP(tensor=pu.tensor, offset=pu.offset,
                                        ap=[pu.ap[0], [256, 2], [1, 128]]),
                            in_=pev, func=EXP, bias=0.0, scale=escale)
                        for k in range(2):
                            nc.scalar.activation(out=pu[:, (2 * k + 1) * 128:(2 * k + 2) * 128],
                                                 in_=pod[k], func=EXP, bias=0.0,
                                                 scale=escale)
                        # mask: zero off-diagonal blocks and self-agent
                        pm = pup.tile([128, 512], BF16, tag="pm", name="pm")
                        nc.vector.tensor_tensor(
                            out=pm,
                            in0=pu,
                            in1=bass.AP(tensor=dmask_sb.tensor, offset=dmask_sb.offset,
                                        ap=[dmask_sb.ap[0], [0, 4], [1, 128]]),
                            op=MULT)
                        if stages < 4:
                            continue
                        # s2: other_bm + Z column per head
                        p2 = ps2.tile([128, 4 * 65], F32, tag="s2", name="s2")
                        for e in range(4):
                            nc.tensor.matmul(out=p2[:, e * 65:(e + 1) * 65],
                                             lhsT=pm[:, e * 128:(e + 1) * 128],
                                             rhs=v2[:, c, e * 65:(e + 1) * 65],
                                             start=True, stop=True)
                        nc.vector.tensor_copy(out=othBM[:, c, :], in_=p2)

                    if stages < 4:
                        continue
                    # normalize: rZ = 1/Z, othN = othBM * rZ (broadcast over d)
                    rz = btp.tile([128, 8, 4], BF16, tag="rz", name="rz")
                    with nc.allow_low_precision(reason="1/Z in bf16 is within tolerance"):
                        nc.vector.reciprocal(
                            out=rz,
                            in_=bass.AP(tensor=othBM.tensor, offset=othBM.offset + D,
                                        ap=[othBM.ap[0], [4 * 65, 8], [65, 4]]))
                    othN = btp.tile([128, 8 * 4 * D], BF16, tag="on", name="on")
                    nc.vector.tensor_tensor(
                        out=othN,
                        in0=bass.AP(tensor=othBM.tensor, offset=othBM.offset,
                                    ap=[othBM.ap[0], [4 * 65, 8], [65, 4], [1, D]]),
                        in1=bass.AP(tensor=rz.tensor, offset=rz.offset,
                                    ap=[rz.ap[0], [4, 8], [1, 4], [0, D]]),
                        op=MULT)

                    # transpose back to feature-major: otherT[ep][:, (i, bt, c, b1)]
                    # one [128,128] transpose covers both heads of an e-pair
                    for ch2 in range(2):
                        for ep in range(2):
                            pt = ptp.tile([128, 512], BF16, tag="tr", name="tr")
                            for c2 in range(4):
                                cg = ch2 * 4 + c2
                                nc.tensor.transpose(
                                    out=pt[:, c2 * 128:(c2 + 1) * 128],
                                    in_=othN[:, (cg * 4 + ep * 2) * D:(cg * 4 + ep * 2 + 2) * D],
                                    identity=identb)
                            # evict to otherT[ep] cols (i*SC + bt*128 + c*16 + b1);
                            # pt cols decode as (c2, i, b1) with the (i*16+b1) rank
                            dst = bass.AP(
                                tensor=otherT[ep].tensor,
                                offset=otherT[ep].offset + bt0 + ch2 * 64,
                                ap=[otherT[ep].ap[0], [16, 4], [SC, 8], [1, 16]])
                            nc.vector.tensor_copy(out=dst, in_=pt)

                # ---------- critic (per agent over the whole superchunk) ----------
                if stages < 5:
                    nc.vector.memset(allq_bm, 0.0)
                for a in range(A if stages >= 5 else 0):
                    h_t = hp.tile([128, 2, SC], BF16, tag="h", name="h")
                    for mt in range(2):
                        ps = pmm.tile([128, SC], F32, tag="mm", name="mm")
                        for kt in range(2):
                            nc.tensor.matmul(out=ps,
                                             lhsT=w_c1_sb[a][:, kt, mt * 128:(mt + 1) * 128],
                                             rhs=s_enc[:, a, kt, :],
                                             start=(kt == 0), stop=False)
                        for kt in range(2):
                            nc.tensor.matmul(out=ps,
                                             lhsT=w_c1_sb[a][:, 2 + kt, mt * 128:(mt + 1) * 128],
                                             rhs=otherT[kt][:, a * SC:(a + 1) * SC],
                                             start=False, stop=(kt == 1))
                        evict_lrelu(ps, h_t[:, mt, :], b_c1_sb[a][:, mt, :])
                    allq_a = hp.tile([16, SC], F32, tag="aq", name="aq", bufs=1)
                    psq = pmm.tile([128, SC], F32, tag="mm", name="mm")
                    for kt in range(2):
                        nc.tensor.matmul(out=psq[0:NACT, :], lhsT=w_c2_sb[a][:, kt, :],
                                         rhs=h_t[:, kt, :],
                                         start=(kt == 0), stop=(kt == 1))
                    nc.scalar.activation(out=allq_a, in_=psq[0:NACT, :], func=IDENT,
                                         bias=b_c2_sb[a], scale=1.0)
                    ptq = pmm.tile([128, SC], F32, tag="mm", name="mm")
                    for bt in range(NBT):
                        nc.tensor.transpose(out=ptq[:, bt * 16:(bt + 1) * 16],
                                            in_=allq_a[:, bt * 128:(bt + 1) * 128],
                                            identity=ident16)
                    # allq_bm cols (bt, a, o)
                    nc.vector.tensor_copy(
                        out=bass.AP(tensor=allq_bm.tensor,
                                    offset=allq_bm.offset + a * NACT,
                                    ap=[allq_bm.ap[0], [128, NBT], [1, NACT]]),
                        in_=ptq[:, 0:NBT * 16])

                # ---------- argmax gather (per b-tile) ----------
                for bt in range(NBT):
                    b0 = sc0 + bt * 128
                    acts_t = btp.tile([128, A, NACT], F32, tag="at", name="at")
                    nc.sync.dma_start(out=acts_t, in_=acts_d[b0:b0 + 128])
                    amax = btp.tile([128, A], F32, tag="am", name="am")
                    nc.vector.tensor_reduce(out=amax, in_=acts_t, axis=X, op=MAX)
                    oh1 = btp.tile([128, A, NACT], F32, tag="oh1", name="oh1", bufs=1)
                    nc.vector.tensor_tensor(
                        out=oh1.rearrange("p a o -> p (a o)"),
                        in0=acts_t.rearrange("p a o -> p (a o)"),
                        in1=bass.AP(tensor=amax.tensor, offset=amax.offset,
                                    ap=[amax.ap[0], [1, A], [0, NACT]]),
                        op=ISGE)
                    # tie-break: keep only the first (lowest-index) max
                    val = btp.tile([128, A, NACT], F32, tag="val", name="val", bufs=1)
                    nc.vector.tensor_tensor(out=val.rearrange("p a o -> p (a o)"),
                                            in0=oh1.rearrange("p a o -> p (a o)"),
                                            in1=w16_sb, op=MULT)
                    m2 = btp.tile([128, A], F32, tag="m2", name="m2")
                    nc.vector.tensor_reduce(out=m2, in_=val, axis=X, op=MAX)
                    oh2 = btp.tile([128, A, NACT], F32, tag="oh2", name="oh2", bufs=1)
                    nc.vector.tensor_tensor(
                        out=oh2.rearrange("p a o -> p (a o)"),
                        in0=val.rearrange("p a o -> p (a o)"),
                        in1=bass.AP(tensor=m2.tensor, offset=m2.offset,
                                    ap=[m2.ap[0], [1, A], [0, NACT]]),
                        op=ISGE)
                    qm = btp.tile([128, A, NACT], F32, tag="qm", name="qm", bufs=1)
                    nc.vector.tensor_tensor(
                        out=qm.rearrange("p a o -> p (a o)"),
                        in0=oh2.rearrange("p a o -> p (a o)"),
                        in1=allq_bm[:, bt * 128:(bt + 1) * 128],
                        op=MULT)
                    q_sb = btp.tile([128, A], F32, tag="qs", name="qs")
                    nc.vector.tensor_reduce(out=q_sb, in_=qm, axis=X, op=ADD)
                    nc.sync.dma_start(out=q_d[b0:b0 + 128], in_=q_sb)

    nc.compile()
    return nc


def _ap64(t, eh, col, SC):
    """64 partitions at base eh; 128 contiguous cols at `col`."""
    row = A * SC  # free elements per partition of the [128, SC*A] tile
    return bass.AP(tensor=t.tensor, offset=t.offset + eh * row + col,
                   ap=[[row, 64], [1, 128]])


def _prep_inputs(states, actions, enc_W, enc_b, s_W, s_b, key_W, sel_W,
                 val_W, val_b, c_W1, c_b1, c_W2, c_b2,
                 b_local=B_LOCAL, n_cores=N_CORES):
    f32 = np.float32
    Bv = b_local * n_cores
    states = states[:, :Bv]
    actions = actions[:, :Bv]
    sa = np.concatenate([states, actions], axis=-1).astype(f32)
    saT = np.zeros((A, KPAD, Bv), dtype=bfloat16)
    saT[:, :IDIM, :] = sa.transpose(0, 2, 1).astype(bfloat16)
    SCW = min(512, b_local)
    n_sc = b_local // SCW
    w_enc = np.zeros((A, KPAD, 2 * H), dtype=bfloat16)
    w_enc[:, :IDIM, :H] = enc_W.astype(bfloat16)
    w_enc[:, :S, H:] = s_W.astype(bfloat16)
    b_enc = np.concatenate([enc_b, s_b], axis=-1).astype(f32).reshape(A, 4, 128, 1)
    w_ks = np.zeros((H, 2 * H), dtype=bfloat16)
    w_ks[:, 0:H] = key_W.transpose(1, 0, 2).reshape(H, H).astype(bfloat16)
    w_ks[:, H:] = sel_W.transpose(1, 0, 2).reshape(H, H).astype(bfloat16)
    w_val = val_W.transpose(1, 0, 2).reshape(H, H).astype(bfloat16)
    bv = np.tile(val_b.reshape(1, -1), (1, 2)).astype(bfloat16)  # doubled: 2 agents/psum
    w_c1 = c_W1.astype(bfloat16)
    b_c1 = c_b1.astype(f32).reshape(A, 2, 128, 1)
    w_c2 = c_W2.astype(bfloat16)
    b_c2 = c_b2.astype(f32).reshape(A, NACT, 1)
    # dmask[(j*16+b2), (i*16+b1)] = (b2==b1) & (i != j)
    dmask = np.zeros((128, 128), dtype=bfloat16)
    for b2 in range(16):
        for j in range(A):
            for i in range(A):
                if i != j:
                    dmask[j * 16 + b2, i * 16 + b2] = 1
    w16 = np.broadcast_to(
        (NACT - np.arange(NACT, dtype=f32))[None, None, :],
        (128, A, NACT)).reshape(128, A * NACT).copy()
    acts_bm = actions.transpose(1, 0, 2).astype(f32)

    shared = dict(w_enc=w_enc, b_enc=b_enc, w_ks=w_ks, w_val=w_val, bv=bv,
                  w_c1=w_c1, b_c1=b_c1, w_c2=w_c2, b_c2=b_c2,
                  dmask=dmask, w16=w16)
    in_maps = []
    for cid in range(n_cores):
        sl = slice(cid * b_local, (cid + 1) * b_local)
        m_ = dict(shared)
        # pack [A, KPAD, b_local] -> [A, n_sc, 128, 4, SCW] so each partition's
        # x-load is one contiguous 4KB run
        sc_ = saT[:, :, sl].reshape(A, 4, 128, n_sc, SCW)
        m_["saT"] = np.ascontiguousarray(sc_.transpose(0, 3, 2, 1, 4))
        m_["acts"] = np.ascontiguousarray(acts_bm[sl])
        in_maps.append(m_)
    return in_maps


_NC_CACHE = {}


def _get_nc(b_local=B_LOCAL):
    if b_local not in _NC_CACHE:
        _NC_CACHE[b_local] = build_bass(b_local)
    return _NC_CACHE[b_local]


def kernel(**inputs):
    inputs = {k: np.asarray(v) for k, v in inputs.items()}
    in_maps = _prep_inputs(**inputs)
    nc = _get_nc()
    res = bass_utils.run_bass_kernel_spmd(
        nc, in_maps, core_ids=list(range(N_CORES)),
        trace=bool(int(os.environ.get("MAAC_TRACE", "0"))))
    q = np.concatenate([r["q"].T for r in res.results], axis=1)  # [A, B]
    if res.exec_time_ns is not None:
        print(f"HW exec time: {res.exec_time_ns} ns")
    return q[:, :, None].astype(np.float32)


# revision 3
# speedup vs baseline: 1.0094x; 1.0094x over previous
"""MAAC critic kernel v2 for Trainium2 — attention on the TensorEngine.

Data-parallel over batch on 8 cores. Per core (b_local=2048), processed in
superchunks of SC=512 (b-tiles of 128, chunks of 16):

  encoder/kvs: feature-major weights-stationary matmuls (bf16).
  stage-1 logits: per (chunk16, head) ONE PE matmul
      out[(b2*8+j), (b1*8+i)] = sum_d keys[d,(b2,j)] * sels[d,(b1,i)]
    — the b2==b1 diagonal blocks are the real logits; the rest is cheap waste.
  softmax stays in that domain: exp on the whole block (ScalarE), then one
    DVE multiply with a constant mask [(b2 j),(b1 i)] = (b2==b1)&(i!=j)
    which zeroes both the off-diagonal garbage and the self-agent term.
  stage-2: out[(b1 i), (d|Z)] = Pm^T @ [V2 | ones] — block-diag structure of
    Pm makes this exact; the appended ones-column yields the softmax
    denominator Z per (b, i) for free.  V2 = vals in [(b0*8+j), ed] layout
    via a dense DRAM round trip (the only partition reshuffle).
  normalize: DVE reciprocal of the Z columns + one broadcast multiply.
  transpose attended values back feature-major via PE transposes, then
  per-agent critic MLP, PE-transpose of q, fp32 one-hot gather with a
  first-index tie-break matching jnp.argmax.
"""
import os
import numpy as np
import ml_dtypes

import concourse.bass as bass
import concourse.tile as tile
import concourse.mybir as mybir
from concourse import bacc
from concourse import bass_utils
from concourse.masks import make_identity

F32 = mybir.dt.float32
BF16 = mybir.dt.bfloat16
bfloat16 = ml_dtypes.bfloat16

A = 8
B = 16384
S = 456
NACT = 16
IDIM = S + NACT
KPAD = 512
H = 256
E = 4
D = 64
ED = E * D               # 256
N_CORES = 8
B_LOCAL = B // N_CORES   # 2048
ALPHA = 0.01


def build_bass(b_local=B_LOCAL, lrelu_act=True, stages=5):
    """stages: 1=enc/kvs, 2=+vals/V2, 3=+s1/softmax, 4=+s2/norm/transpose, 5=all."""
    nc = bacc.Bacc("TRN2", target_bir_lowering=False, debug=False)
    SC = min(512, b_local)       # superchunk width (free dim of big matmuls)
    n_sc = b_local // SC
    NBT = SC // 128              # b-tiles per superchunk
    LRELU = mybir.ActivationFunctionType.Lrelu
    EXP = mybir.ActivationFunctionType.Exp
    IDENT = mybir.ActivationFunctionType.Identity
    MULT = mybir.AluOpType.mult
    ADD = mybir.AluOpType.add
    ISGE = mybir.AluOpType.is_ge
    MAX = mybir.AluOpType.max
    X = mybir.AxisListType.X

    # ---- DRAM I/O ----
    saT_d = nc.dram_tensor("saT", [A, b_local // min(512, b_local), 128, 4,
                                   min(512, b_local)], BF16, kind="ExternalInput")
    acts_d = nc.dram_tensor("acts", [b_local, A, NACT], F32, kind="ExternalInput")
    w_enc_d = nc.dram_tensor("w_enc", [A, KPAD, 2 * H], BF16, kind="ExternalInput")
    b_enc_d = nc.dram_tensor("b_enc", [A, 4, 128, 1], F32, kind="ExternalInput")
    w_ks_d = nc.dram_tensor("w_ks", [H, 2 * H], BF16, kind="ExternalInput")
    w_val_d = nc.dram_tensor("w_val", [H, ED], BF16, kind="ExternalInput")
    bv_d = nc.dram_tensor("bv", [1, 2 * ED], BF16, kind="ExternalInput")
    w_c1_d = nc.dram_tensor("w_c1", [A, 2 * H, H], BF16, kind="ExternalInput")
    b_c1_d = nc.dram_tensor("b_c1", [A, 2, 128, 1], F32, kind="ExternalInput")
    w_c2_d = nc.dram_tensor("w_c2", [A, H, NACT], BF16, kind="ExternalInput")
    b_c2_d = nc.dram_tensor("b_c2", [A, NACT, 1], F32, kind="ExternalInput")
    dmask_d = nc.dram_tensor("dmask", [128, 128], BF16, kind="ExternalInput")
    w16_d = nc.dram_tensor("w16", [128, A * NACT], F32, kind="ExternalInput")
    q_d = nc.dram_tensor("q", [b_local, A], F32, kind="ExternalOutput")

    with tile.TileContext(nc) as tc:
        import contextlib
        with contextlib.ExitStack() as ctx:
            wp = ctx.enter_context(tc.tile_pool(name="wp", bufs=1))
            xp = ctx.enter_context(tc.tile_pool(name="xp", bufs=2))
            scp = ctx.enter_context(tc.tile_pool(name="scp", bufs=1))
            btp = ctx.enter_context(tc.tile_pool(name="btp", bufs=2))
            pup = ctx.enter_context(tc.tile_pool(name="pup", bufs=2))
            hp = ctx.enter_context(tc.tile_pool(name="hp", bufs=2))
            pmm = ctx.enter_context(tc.tile_pool(name="pmm", bufs=2, space="PSUM"))
            ps1 = ctx.enter_context(tc.tile_pool(name="ps1", bufs=1, space="PSUM"))
            ps2 = ctx.enter_context(tc.tile_pool(name="ps2", bufs=1, space="PSUM"))
            ptp = ctx.enter_context(tc.tile_pool(name="ptp", bufs=2, space="PSUM"))
            dp = ctx.enter_context(tc.tile_pool(name="dp", bufs=2, space="DRAM"))

            # ---- resident weights ----
            w_enc_sb, b_enc_sb = [], []
            w_c1_sb, b_c1_sb, w_c2_sb, b_c2_sb = [], [], [], []
            for a in range(A):
                w = wp.tile([128, 4, 2 * H], BF16, name=f"w_enc{a}")
                nc.sync.dma_start(out=w, in_=w_enc_d[a].rearrange("(kt p) m -> p kt m", p=128))
                w_enc_sb.append(w)
                bt_ = wp.tile([128, 4, 1], F32, name=f"b_enc{a}")
                nc.sync.dma_start(out=bt_, in_=b_enc_d[a].rearrange("kt p one -> p kt one"))
                b_enc_sb.append(bt_)
                w1 = wp.tile([128, 4, H], BF16, name=f"w_c1{a}")
                nc.sync.dma_start(out=w1, in_=w_c1_d[a].rearrange("(kt p) m -> p kt m", p=128))
                w_c1_sb.append(w1)
                b1 = wp.tile([128, 2, 1], F32, name=f"b_c1{a}")
                nc.sync.dma_start(out=b1, in_=b_c1_d[a].rearrange("kt p one -> p kt one"))
                b_c1_sb.append(b1)
                w2 = wp.tile([128, 2, NACT], BF16, name=f"w_c2{a}")
                nc.sync.dma_start(out=w2, in_=w_c2_d[a].rearrange("(kt p) m -> p kt m", p=128))
                w_c2_sb.append(w2)
                b2 = wp.tile([NACT, 1], F32, name=f"b_c2{a}")
                nc.sync.dma_start(out=b2, in_=b_c2_d[a])
                b_c2_sb.append(b2)
            w_ks_sb = wp.tile([128, 2, 2 * H], BF16, name="w_ks")
            nc.sync.dma_start(out=w_ks_sb, in_=w_ks_d.rearrange("(kt p) m -> p kt m", p=128))
            w_val_sb = wp.tile([128, 2, ED], BF16, name="w_val")
            nc.sync.dma_start(out=w_val_sb, in_=w_val_d.rearrange("(kt p) m -> p kt m", p=128))
            bv2_sb = wp.tile([1, 2 * ED], BF16, name="bv2")
            nc.sync.dma_start(out=bv2_sb, in_=bv_d[:, :])
            ones_sb = wp.tile([1, 128], BF16, name="ones")
            nc.vector.memset(ones_sb, 1.0)
            dmask_sb = wp.tile([128, 128], BF16, name="dmask")
            nc.sync.dma_start(out=dmask_sb, in_=dmask_d[:, :])
            w16_sb = wp.tile([128, A * NACT], F32, name="w16")
            nc.sync.dma_start(out=w16_sb, in_=w16_d[:, :])
            identb = wp.tile([128, 128], BF16, name="identb")
            make_identity(nc, identb)
            ident16 = wp.tile([16, 16], F32, name="ident16")
            make_identity(nc, ident16)

            RELU = mybir.ActivationFunctionType.Relu
            b2_cache = {}

            def evict_lrelu(psum_ap, out_ap, bias):
                """out = lrelu(psum + bias); bias is a [128,1] AP or None.

                Table-free on ScalarE (Identity w/ bias) so the Exp act-table
                stays resident; the leak is one DVE max(y, alpha*y)."""
                if lrelu_act:
                    n = psum_ap.shape[-1]
                    tmp = pup.tile([128, SC], BF16, tag="lr", name="lr", bufs=3)
                    tn = tmp[:, :n] if n != SC else tmp
                    nc.scalar.activation(out=tn, in_=psum_ap, func=IDENT,
                                         bias=bias if bias is not None else 0.0,
                                         scale=1.0)
                    nc.vector.scalar_tensor_tensor(
                        out=out_ap, in0=tn, scalar=ALPHA, op0=MULT,
                        in1=tn, op1=mybir.AluOpType.max)
                    return
                # CoreSim fallback: lrelu(y) = y + relu(-(1-a)*y), y = x + b
                n = psum_ap.shape[-1]
                r = xp.tile([128, SC], F32, tag="lrtmp", name="lrtmp")
                rr = r[:, :n] if n != SC else r
                bias2 = 0.0
                if bias is not None:
                    key = (bias.tensor.name, bias.offset)
                    if key not in b2_cache:
                        b2 = wp.tile([128, 1], F32, name=f"b2_{len(b2_cache)}")
                        nc.vector.tensor_scalar_mul(out=b2, in0=bias,
                                                    scalar1=-(1.0 - ALPHA))
                        b2_cache[key] = b2
                    bias2 = b2_cache[key]
                nc.scalar.activation(out=rr, in_=psum_ap, func=RELU,
                                     bias=bias2, scale=-(1.0 - ALPHA))
                nc.vector.scalar_tensor_tensor(
                    out=out_ap, in0=psum_ap,
                    scalar=bias if bias is not None else 0.0,
                    op0=ADD, in1=rr, op1=ADD)

            for sc in range(n_sc):
                sc0 = sc * SC
                sa_enc = scp.tile([128, A, 2, SC], BF16, tag="sa_enc", name="sa_enc")
                s_enc = scp.tile([128, A, 2, SC], BF16, tag="s_enc", name="s_enc",
                                 bufs=2)
                # keys/sels stored feature-major with cols (chunk, b2, j) so the
                # s1 matmul operands are contiguous 128-col slices
                keys_fm = [scp.tile([128, SC * A], BF16, tag=f"keys{t}", name=f"keys{t}") for t in range(2)]
                sels_fm = [scp.tile([128, SC * A], BF16, tag=f"sels{t}", name=f"sels{t}") for t in range(2)]
                otherT = [scp.tile([128, A * SC], BF16, tag=f"oT{t}", name=f"oT{t}") for t in range(2)]
                allq_bm = scp.tile([128, NBT * 128], F32, tag="allq_bm", name="allq_bm")

                # ---------- encoder + keys/sels (feature-major) ----------
                for a in range(A):
                    x = xp.tile([128, 4, SC], BF16, tag="x", name="x")
                    nc.sync.dma_start(out=x, in_=saT_d[a, sc])
                    for mt in range(4):
                        ps = pmm.tile([128, SC], F32, tag="mm", name="mm")
                        for kt in range(4):
                            nc.tensor.matmul(out=ps,
                                             lhsT=w_enc_sb[a][:, kt, mt * 128:(mt + 1) * 128],
                                             rhs=x[:, kt, :],
                                             start=(kt == 0), stop=(kt == 3))
                        dst = sa_enc[:, a, mt, :] if mt < 2 else s_enc[:, a, mt - 2, :]
                        evict_lrelu(ps, dst, b_enc_sb[a][:, mt, :])
                    # keys (from sa_enc) and sels (from s_enc), feature-major,
                    # evicted into (chunk, j=a, b2) interleaved column order
                    # (16-wide contiguous runs; attention row rank = j*16+b2)
                    def ks_dst(t, a):
                        return bass.AP(tensor=t.tensor, offset=t.offset + a * 16,
                                       ap=[t.ap[0], [128, SC // 16], [1, 16]])
                    for et in range(2):
                        ps = pmm.tile([128, SC], F32, tag="mm", name="mm")
                        for kt in range(2):
                            nc.tensor.matmul(out=ps,
                                             lhsT=w_ks_sb[:, kt, et * 128:(et + 1) * 128],
                                             rhs=sa_enc[:, a, kt, :],
                                             start=(kt == 0), stop=(kt == 1))
                        nc.vector.tensor_copy(out=ks_dst(keys_fm[et], a), in_=ps)
                        ps = pmm.tile([128, SC], F32, tag="mm", name="mm")
                        for kt in range(2):
                            nc.tensor.matmul(out=ps,
                                             lhsT=w_ks_sb[:, kt, H + et * 128:H + (et + 1) * 128],
                                             rhs=s_enc[:, a, kt, :],
                                             start=(kt == 0), stop=(kt == 1))
                        nc.vector.tensor_copy(out=ks_dst(sels_fm[et], a), in_=ps)

                # ---------- per b-tile: vals, V2 bounce, s1, softmax, s2 ----------
                for bt in range(NBT if stages >= 2 else 0):
                    bt0 = bt * 128
                    # vals b-major [128b, (j, ed)]
                    vals_bm = btp.tile([128, A, ED], BF16, tag="vb", name="vb", bufs=1)
                    for a in range(A):
                        ps = pmm.tile([128, SC], F32, tag="mm", name="mm")
                        for kt in range(2):
                            nc.tensor.matmul(out=ps[:, 0:ED],
                                             lhsT=sa_enc[:, a, kt, bt0:bt0 + 128],
                                             rhs=w_val_sb[:, kt, :],
                                             start=(kt == 0), stop=False)
                        nc.tensor.matmul(out=ps[:, 0:ED], lhsT=ones_sb,
                                         rhs=bv2_sb[:, 0:ED], start=False, stop=True)
                        evict_lrelu(ps[:, 0:ED], vals_bm[:, a, :], None)
                    # V2 via DRAM bounce: vdram [c][j][b0][ed]; row rank = j*16+b0
                    vdram = dp.tile([8, A, 16, ED], BF16, tag="vd", name="vd")
                    for j in range(A):
                        nc.sync.dma_start(
                            out=bass.AP(tensor=vdram.tensor, offset=vdram.offset + j * 16 * ED,
                                        ap=[[A * 16 * ED, 8], [ED, 16], [1, ED]]),
                            in_=vals_bm[:, j, :])
                    V2C = 4 * 65  # 260 cols per chunk: 4 x (64 vals + ones col)
                    v2 = btp.tile([128, 8, V2C], BF16, tag="v2", name="v2")
                    for c in range(8):
                        src = bass.AP(tensor=vdram.tensor, offset=vdram.offset + c * A * 16 * ED,
                                      ap=[[ED, 128], [D, 4], [1, D]])
                        dst = bass.AP(tensor=v2.tensor, offset=v2.offset + c * V2C,
                                      ap=[v2.ap[0], [65, 4], [1, D]])
                        nc.gpsimd.dma_start(out=dst, in_=src)
                    # ones columns at e*65+64
                    nc.vector.memset(
                        bass.AP(tensor=v2.tensor, offset=v2.offset + D,
                                ap=[v2.ap[0], [V2C, 8], [65, 4]]), 1.0)

                    if stages < 3:
                        continue
                    othBM = btp.tile([128, 8, 4 * 65], BF16, tag="ob", name="ob",
                                     bufs=1)
                    for c in range(8):
                        # s1: logits c2-block per head.  Base-64 operands
                        # (odd heads) must target a psum tile at bank offset 0,
                        # so e1/e3 get their own tiles.
                        cg = bt * 8 + c  # chunk index within superchunk
                        pev = ps1.tile([128, 256], F32, tag="s1e", name="s1e")
                        pod = [ps1.tile([128, 128], F32, tag=f"s1o{k}", name=f"s1o{k}")
                               for k in range(2)]
                        for e in range(4):
                            et, eh = e // 2, (e % 2) * 64
                            lhsT = _ap64(keys_fm[et], eh, cg * 128, SC)
                            rhs = _ap64(sels_fm[et], eh, cg * 128, SC)
                            out = (pev[:, (e // 2) * 128:(e // 2 + 1) * 128]
                                   if e % 2 == 0 else pod[e // 2])
                            nc.tensor.matmul(out=out, lhsT=lhsT, rhs=rhs,
                                             start=True, stop=True)
                        # exp (scale 1/sqrt(D)); pu cols stay in (e, b1, i) order
                        pu = pup.tile([128, 512], BF16, tag="pu", name="pu")
                        escale = 1.0 / np.sqrt(np.float32(D))
                        nc.scalar.activation(
                            out=bass.AP(tensor=pu.tensor, offset=pu.offset,
                                        ap=[pu.ap[0], [256, 2], [1, 128]]),
                            in_=pev, func=EXP, bias=0.0, scale=escale)
                        for k in range(2):
                            nc.scalar.activation(out=pu[:, (2 * k + 1) * 128:(2 * k + 2) * 128],
                                                 in_=pod[k], func=EXP, bias=0.0,
                                                 scale=escale)
                        # mask: zero off-diagonal blocks and self-agent
                        pm = pup.tile([128, 512], BF16, tag="pm", name="pm")
                        nc.vector.tensor_tensor(
                            out=pm,
                            in0=pu,
                            in1=bass.AP(tensor=dmask_sb.tensor, offset=dmask_sb.offset,
                                        ap=[dmask_sb.ap[0], [0, 4], [1, 128]]),
                            op=MULT)
                        if stages < 4:
                            continue
                        # s2: other_bm + Z column per head
                        p2 = ps2.tile([128, 4 * 65], F32, tag="s2", name="s2")
                        for e in range(4):
                            nc.tensor.matmul(out=p2[:, e * 65:(e + 1) * 65],
                                             lhsT=pm[:, e * 128:(e + 1) * 128],
                                             rhs=v2[:, c, e * 65:(e + 1) * 65],
                                             start=True, stop=True)
                        nc.vector.tensor_copy(out=othBM[:, c, :], in_=p2)

                    if stages < 4:
                        continue
                    # normalize: rZ = 1/Z, othN = othBM * rZ (broadcast over d)
                    rz = btp.tile([128, 8, 4], BF16, tag="rz", name="rz")
                    with nc.allow_low_precision(reason="1/Z in bf16 is within tolerance"):
                        nc.vector.reciprocal(
                            out=rz,
                            in_=bass.AP(tensor=othBM.tensor, offset=othBM.offset + D,
                                        ap=[othBM.ap[0], [4 * 65, 8], [65, 4]]))
                    othN = btp.tile([128, 8 * 4 * D], BF16, tag="on", name="on")
                    nc.vector.tensor_tensor(
                        out=othN,
                        in0=bass.AP(tensor=othBM.tensor, offset=othBM.offset,
                                    ap=[othBM.ap[0], [4 * 65, 8], [65, 4], [1, D]]),
                        in1=bass.AP(tensor=rz.tensor, offset=rz.offset,
                                    ap=[rz.ap[0], [4, 8], [1, 4], [0, D]]),
                        op=MULT)

                    # transpose back to feature-major: otherT[ep][:, (i, bt, c, b1)]
                    # one [128,128] transpose covers both heads of an e-pair
                    for ch2 in range(2):
                        for ep in range(2):
                            pt = ptp.tile([128, 512], BF16, tag="tr", name="tr")
                            for c2 in range(4):
                                cg = ch2 * 4 + c2
                                nc.tensor.transpose(
                                    out=pt[:, c2 * 128:(c2 + 1) * 128],
                                    in_=othN[:, (cg * 4 + ep * 2) * D:(cg * 4 + ep * 2 + 2) * D],
                                    identity=identb)
                            # evict to otherT[ep] cols (i*SC + bt*128 + c*16 + b1);
                            # pt cols decode as (c2, i, b1) with the (i*16+b1) rank
                            dst = bass.AP(
                                tensor=otherT[ep].tensor,
                                offset=otherT[ep].offset + bt0 + ch2 * 64,
                                ap=[otherT[ep].ap[0], [16, 4], [SC, 8], [1, 16]])
                            nc.vector.tensor_copy(out=dst, in_=pt)

                # ---------- critic (per agent over the whole superchunk) ----------
                if stages < 5:
                    nc.vector.memset(allq_bm, 0.0)
                for a in range(A if stages >= 5 else 0):
                    h_t = hp.tile([128, 2, SC], BF16, tag="h", name="h")
                    for mt in range(2):
                        ps = pmm.tile([128, SC], F32, tag="mm", name="mm")
                        for kt in range(2):
                            nc.tensor.matmul(out=ps,
                                             lhsT=w_c1_sb[a][:, kt, mt * 128:(mt + 1) * 128],
                                             rhs=s_enc[:, a, kt, :],
                                             start=(kt == 0), stop=False)
                        for kt in range(2):
                            nc.tensor.matmul(out=ps,
                                             lhsT=w_c1_sb[a][:, 2 + kt, mt * 128:(mt + 1) * 128],
                                             rhs=otherT[kt][:, a * SC:(a + 1) * SC],
                                             start=False, stop=(kt == 1))
                        evict_lrelu(ps, h_t[:, mt, :], b_c1_sb[a][:, mt, :])
                    allq_a = hp.tile([16, SC], F32, tag="aq", name="aq", bufs=1)
                    psq = pmm.tile([128, SC], F32, tag="mm", name="mm")
                    for kt in range(2):
                        nc.tensor.matmul(out=psq[0:NACT, :], lhsT=w_c2_sb[a][:, kt, :],
                                         rhs=h_t[:, kt, :],
                                         start=(kt == 0), stop=(kt == 1))
                    nc.scalar.activation(out=allq_a, in_=psq[0:NACT, :], func=IDENT,
                                         bias=b_c2_sb[a], scale=1.0)
                    ptq = pmm.tile([128, SC], F32, tag="mm", name="mm")
                    for bt in range(NBT):
                        nc.tensor.transpose(out=ptq[:, bt * 16:(bt + 1) * 16],
                                            in_=allq_a[:, bt * 128:(bt + 1) * 128],
                                            identity=ident16)
                    # allq_bm cols (bt, a, o)
                    nc.vector.tensor_copy(
                        out=bass.AP(tensor=allq_bm.tensor,
                                    offset=allq_bm.offset + a * NACT,
                                    ap=[allq_bm.ap[0], [128, NBT], [1, NACT]]),
                        in_=ptq[:, 0:NBT * 16])

                # ---------- argmax gather (per b-tile) ----------
                for bt in range(NBT):
                    b0 = sc0 + bt * 128
                    acts_t = btp.tile([128, A, NACT], F32, tag="at", name="at")
                    nc.sync.dma_start(out=acts_t, in_=acts_d[b0:b0 + 128])
                    amax = btp.tile([128, A], F32, tag="am", name="am")
                    nc.vector.tensor_reduce(out=amax, in_=acts_t, axis=X, op=MAX)
                    oh1 = btp.tile([128, A, NACT], F32, tag="oh1", name="oh1", bufs=1)
                    nc.vector.tensor_tensor(
                        out=oh1.rearrange("p a o -> p (a o)"),
                        in0=acts_t.rearrange("p a o -> p (a o)"),
                        in1=bass.AP(tensor=amax.tensor, offset=amax.offset,
                                    ap=[amax.ap[0], [1, A], [0, NACT]]),
                        op=ISGE)
                    # tie-break: keep only the first (lowest-index) max
                    val = btp.tile([128, A, NACT], F32, tag="val", name="val", bufs=1)
                    nc.vector.tensor_tensor(out=val.rearrange("p a o -> p (a o)"),
                                            in0=oh1.rearrange("p a o -> p (a o)"),
                                            in1=w16_sb, op=MULT)
                    m2 = btp.tile([128, A], F32, tag="m2", name="m2")
                    nc.vector.tensor_reduce(out=m2, in_=val, axis=X, op=MAX)
                    oh2 = btp.tile([128, A, NACT], F32, tag="oh2", name="oh2", bufs=1)
                    nc.vector.tensor_tensor(
                        out=oh2.rearrange("p a o -> p (a o)"),
                        in0=val.rearrange("p a o -> p (a o)"),
                        in1=bass.AP(tensor=m2.tensor, offset=m2.offset,
                                    ap=[m2.ap[0], [1, A], [0, NACT]]),
                        op=ISGE)
                    qm = btp.tile([128, A, NACT], F32, tag="qm", name="qm", bufs=1)
                    nc.vector.tensor_tensor(
                        out=qm.rearrange("p a o -> p (a o)"),
                        in0=oh2.rearrange("p a o -> p (a o)"),
                        in1=allq_bm[:, bt * 128:(bt + 1) * 128],
                        op=MULT)
                    q_sb = btp.tile([128, A], F32, tag="qs", name="qs")
                    nc.vector.tensor_reduce(out=q_sb, in_=qm, axis=X, op=ADD)
                    nc.sync.dma_start(out=q_d[b0:b0 + 128], in_=q_sb)

    nc.compile()
    return nc


def _ap64(t, eh, col, SC):
    """64 partitions at base eh; 128 contiguous cols at `col`."""
    row = A * SC  # free elements per partition of the [128, SC*A] tile
    return bass.AP(tensor=t.tensor, offset=t.offset + eh * row + col,
                   ap=[[row, 64], [1, 128]])


def _prep_inputs(states, actions, enc_W, enc_b, s_W, s_b, key_W, sel_W,
                 val_W, val_b, c_W1, c_b1, c_W2, c_b2,
                 b_local=B_LOCAL, n_cores=N_CORES):
    f32 = np.float32
    Bv = b_local * n_cores
    states = states[:, :Bv]
    actions = actions[:, :Bv]
    sa = np.concatenate([states, actions], axis=-1).astype(f32)
    saT = np.zeros((A, KPAD, Bv), dtype=bfloat16)
    saT[:, :IDIM, :] = sa.transpose(0, 2, 1).astype(bfloat16)
    SCW = min(512, b_local)
    n_sc = b_local // SCW
    w_enc = np.zeros((A, KPAD, 2 * H), dtype=bfloat16)
    w_enc[:, :IDIM, :H] = enc_W.astype(bfloat16)
    w_enc[:, :S, H:] = s_W.astype(bfloat16)
    b_enc = np.concatenate([enc_b, s_b], axis=-1).astype(f32).reshape(A, 4, 128, 1)
    w_ks = np.zeros((H, 2 * H), dtype=bfloat16)
    w_ks[:, 0:H] = key_W.transpose(1, 0, 2).reshape(H, H).astype(bfloat16)
    w_ks[:, H:] = sel_W.transpose(1, 0, 2).reshape(H, H).astype(bfloat16)
    w_val = val_W.transpose(1, 0, 2).reshape(H, H).astype(bfloat16)
    bv = np.tile(val_b.reshape(1, -1), (1, 2)).astype(bfloat16)  # doubled: 2 agents/psum
    w_c1 = c_W1.astype(bfloat16)
    b_c1 = c_b1.astype(f32).reshape(A, 2, 128, 1)
    w_c2 = c_W2.astype(bfloat16)
    b_c2 = c_b2.astype(f32).reshape(A, NACT, 1)
    # dmask[(j*16+b2), (i*16+b1)] = (b2==b1) & (i != j)
    dmask = np.zeros((128, 128), dtype=bfloat16)
    for b2 in range(16):
        for j in range(A):
            for i in range(A):
                if i != j:
                    dmask[j * 16 + b2, i * 16 + b2] = 1
    w16 = np.broadcast_to(
        (NACT - np.arange(NACT, dtype=f32))[None, None, :],
        (128, A, NACT)).reshape(128, A * NACT).copy()
    acts_bm = actions.transpose(1, 0, 2).astype(f32)

    shared = dict(w_enc=w_enc, b_enc=b_enc, w_ks=w_ks, w_val=w_val, bv=bv,
                  w_c1=w_c1, b_c1=b_c1, w_c2=w_c2, b_c2=b_c2,
                  dmask=dmask, w16=w16)
    in_maps = []
    for cid in range(n_cores):
        sl = slice(cid * b_local, (cid + 1) * b_local)
        m_ = dict(shared)
        # pack [A, KPAD, b_local] -> [A, n_sc, 128, 4, SCW] so each partition's
        # x-load is one contiguous 4KB run
        sc_ = saT[:, :, sl].reshape(A, 4, 128, n_sc, SCW)
        m_["saT"] = np.ascontiguousarray(sc_.transpose(0, 3, 2, 1, 4))
        m_["acts"] = np.ascontiguousarray(acts_bm[sl])
        in_maps.append(m_)
    return in_maps


_NC_CACHE = {}


def _get_nc(b_local=B_LOCAL):
    if b_local not in _NC_CACHE:
        _NC_CACHE[b_local] = build_bass(b_local)
    return _NC_CACHE[b_local]


def kernel(**inputs):
    inputs = {k: np.asarray(v) for k, v in inputs.items()}
    in_maps = _prep_inputs(**inputs)
    nc = _get_nc()
    res = bass_utils.run_bass_kernel_spmd(
        nc, in_maps, core_ids=list(range(N_CORES)),
        trace=bool(int(os.environ.get("MAAC_TRACE", "0"))))
    q = np.concatenate([r["q"].T for r in res.results], axis=1)  # [A, B]
    if res.exec_time_ns is not None:
        print(f"HW exec time: {res.exec_time_ns} ns")
    return q[:, :, None].astype(np.float32)


# revision 4
# speedup vs baseline: 1.0150x; 1.0055x over previous
"""MAAC critic kernel v2 for Trainium2 — attention on the TensorEngine.

Data-parallel over batch on 8 cores. Per core (b_local=2048), processed in
superchunks of SC=512 (b-tiles of 128, chunks of 16):

  encoder/kvs: feature-major weights-stationary matmuls (bf16).
  stage-1 logits: per (chunk16, head) ONE PE matmul
      out[(b2*8+j), (b1*8+i)] = sum_d keys[d,(b2,j)] * sels[d,(b1,i)]
    — the b2==b1 diagonal blocks are the real logits; the rest is cheap waste.
  softmax stays in that domain: exp on the whole block (ScalarE), then one
    DVE multiply with a constant mask [(b2 j),(b1 i)] = (b2==b1)&(i!=j)
    which zeroes both the off-diagonal garbage and the self-agent term.
  stage-2: out[(b1 i), (d|Z)] = Pm^T @ [V2 | ones] — block-diag structure of
    Pm makes this exact; the appended ones-column yields the softmax
    denominator Z per (b, i) for free.  V2 = vals in [(b0*8+j), ed] layout
    via a dense DRAM round trip (the only partition reshuffle).
  normalize: DVE reciprocal of the Z columns + one broadcast multiply.
  transpose attended values back feature-major via PE transposes, then
  per-agent critic MLP, PE-transpose of q, fp32 one-hot gather with a
  first-index tie-break matching jnp.argmax.
"""
import os
import numpy as np
import ml_dtypes

import concourse.bass as bass
import concourse.tile as tile
import concourse.mybir as mybir
from concourse import bacc
from concourse import bass_utils
from concourse.masks import make_identity

F32 = mybir.dt.float32
BF16 = mybir.dt.bfloat16
bfloat16 = ml_dtypes.bfloat16

A = 8
B = 16384
S = 456
NACT = 16
IDIM = S + NACT
KPAD = 512
H = 256
E = 4
D = 64
ED = E * D               # 256
N_CORES = 8
B_LOCAL = B // N_CORES   # 2048
ALPHA = 0.01


def build_bass(b_local=B_LOCAL, lrelu_act=True, stages=5):
    """stages: 1=enc/kvs, 2=+vals/V2, 3=+s1/softmax, 4=+s2/norm/transpose, 5=all."""
    nc = bacc.Bacc("TRN2", target_bir_lowering=False, debug=False)
    SC = min(512, b_local)       # superchunk width (free dim of big matmuls)
    n_sc = b_local // SC
    NBT = SC // 128              # b-tiles per superchunk
    LRELU = mybir.ActivationFunctionType.Lrelu
    EXP = mybir.ActivationFunctionType.Exp
    IDENT = mybir.ActivationFunctionType.Identity
    MULT = mybir.AluOpType.mult
    ADD = mybir.AluOpType.add
    ISGE = mybir.AluOpType.is_ge
    MAX = mybir.AluOpType.max
    X = mybir.AxisListType.X

    # ---- DRAM I/O ----
    saT_d = nc.dram_tensor("saT", [A, b_local // min(512, b_local), 128, 4,
                                   min(512, b_local)], BF16, kind="ExternalInput")
    acts_d = nc.dram_tensor("acts", [b_local, A, NACT], F32, kind="ExternalInput")
    w_enc_d = nc.dram_tensor("w_enc", [A, KPAD, 2 * H], BF16, kind="ExternalInput")
    b_enc_d = nc.dram_tensor("b_enc", [A, 4, 128, 1], F32, kind="ExternalInput")
    w_ks_d = nc.dram_tensor("w_ks", [H, 2 * H], BF16, kind="ExternalInput")
    w_val_d = nc.dram_tensor("w_val", [H, ED], BF16, kind="ExternalInput")
    bv_d = nc.dram_tensor("bv", [1, 2 * ED], BF16, kind="ExternalInput")
    w_c1_d = nc.dram_tensor("w_c1", [A, 2 * H, H], BF16, kind="ExternalInput")
    b_c1_d = nc.dram_tensor("b_c1", [A, 2, 128, 1], F32, kind="ExternalInput")
    w_c2_d = nc.dram_tensor("w_c2", [A, H, NACT], BF16, kind="ExternalInput")
    b_c2_d = nc.dram_tensor("b_c2", [A, NACT, 1], F32, kind="ExternalInput")
    dmask_d = nc.dram_tensor("dmask", [128, 128], BF16, kind="ExternalInput")
    w16_d = nc.dram_tensor("w16", [128, A * NACT], F32, kind="ExternalInput")
    q_d = nc.dram_tensor("q", [b_local, A], F32, kind="ExternalOutput")

    with tile.TileContext(nc) as tc:
        import contextlib
        with contextlib.ExitStack() as ctx:
            wp = ctx.enter_context(tc.tile_pool(name="wp", bufs=1))
            xp = ctx.enter_context(tc.tile_pool(name="xp", bufs=2))
            scp = ctx.enter_context(tc.tile_pool(name="scp", bufs=1))
            btp = ctx.enter_context(tc.tile_pool(name="btp", bufs=2))
            pup = ctx.enter_context(tc.tile_pool(name="pup", bufs=2))
            hp = ctx.enter_context(tc.tile_pool(name="hp", bufs=2))
            pmm = ctx.enter_context(tc.tile_pool(name="pmm", bufs=2, space="PSUM"))
            ps1 = ctx.enter_context(tc.tile_pool(name="ps1", bufs=1, space="PSUM"))
            ps2 = ctx.enter_context(tc.tile_pool(name="ps2", bufs=1, space="PSUM"))
            ptp = ctx.enter_context(tc.tile_pool(name="ptp", bufs=2, space="PSUM"))
            dp = ctx.enter_context(tc.tile_pool(name="dp", bufs=2, space="DRAM"))

            # ---- resident weights ----
            w_enc_sb, b_enc_sb = [], []
            w_c1_sb, b_c1_sb, w_c2_sb, b_c2_sb = [], [], [], []
            for a in range(A):
                w = wp.tile([128, 4, 2 * H], BF16, name=f"w_enc{a}")
                (nc.sync if a == 0 else nc.scalar).dma_start(
                    out=w, in_=w_enc_d[a].rearrange("(kt p) m -> p kt m", p=128))
                w_enc_sb.append(w)
                bt_ = wp.tile([128, 4, 1], F32, name=f"b_enc{a}")
                nc.sync.dma_start(out=bt_, in_=b_enc_d[a].rearrange("kt p one -> p kt one"))
                b_enc_sb.append(bt_)
                w1 = wp.tile([128, 4, H], BF16, name=f"w_c1{a}")
                nc.scalar.dma_start(out=w1, in_=w_c1_d[a].rearrange("(kt p) m -> p kt m", p=128))
                w_c1_sb.append(w1)
                b1 = wp.tile([128, 2, 1], F32, name=f"b_c1{a}")
                nc.scalar.dma_start(out=b1, in_=b_c1_d[a].rearrange("kt p one -> p kt one"))
                b_c1_sb.append(b1)
                w2 = wp.tile([128, 2, NACT], BF16, name=f"w_c2{a}")
                nc.scalar.dma_start(out=w2, in_=w_c2_d[a].rearrange("(kt p) m -> p kt m", p=128))
                w_c2_sb.append(w2)
                b2 = wp.tile([NACT, 1], F32, name=f"b_c2{a}")
                nc.scalar.dma_start(out=b2, in_=b_c2_d[a])
                b_c2_sb.append(b2)
            w_ks_sb = wp.tile([128, 2, 2 * H], BF16, name="w_ks")
            nc.sync.dma_start(out=w_ks_sb, in_=w_ks_d.rearrange("(kt p) m -> p kt m", p=128))
            w_val_sb = wp.tile([128, 2, ED], BF16, name="w_val")
            nc.sync.dma_start(out=w_val_sb, in_=w_val_d.rearrange("(kt p) m -> p kt m", p=128))
            bv2_sb = wp.tile([1, 2 * ED], BF16, name="bv2")
            nc.sync.dma_start(out=bv2_sb, in_=bv_d[:, :])
            ones_sb = wp.tile([1, 128], BF16, name="ones")
            nc.vector.memset(ones_sb, 1.0)
            dmask_sb = wp.tile([128, 128], BF16, name="dmask")
            nc.sync.dma_start(out=dmask_sb, in_=dmask_d[:, :])
            w16_sb = wp.tile([128, A * NACT], F32, name="w16")
            nc.sync.dma_start(out=w16_sb, in_=w16_d[:, :])
            identb = wp.tile([128, 128], BF16, name="identb")
            make_identity(nc, identb)
            ident16 = wp.tile([16, 16], F32, name="ident16")
            make_identity(nc, ident16)

            RELU = mybir.ActivationFunctionType.Relu
            b2_cache = {}

            def evict_lrelu(psum_ap, out_ap, bias, eng=None):
                """out = lrelu(psum + bias); bias is a [128,1] AP or None.

                Table-free on ScalarE (Identity w/ bias) so the Exp act-table
                stays resident; the leak is one max(y, alpha*y) on eng."""
                if lrelu_act:
                    n = psum_ap.shape[-1]
                    tmp = pup.tile([128, SC], BF16, tag="lr", name="lr", bufs=3)
                    tn = tmp[:, :n] if n != SC else tmp
                    nc.scalar.activation(out=tn, in_=psum_ap, func=IDENT,
                                         bias=bias if bias is not None else 0.0,
                                         scale=1.0)
                    (eng or nc.vector).scalar_tensor_tensor(
                        out=out_ap, in0=tn, scalar=ALPHA, op0=MULT,
                        in1=tn, op1=mybir.AluOpType.max)
                    return
                # CoreSim fallback: lrelu(y) = y + relu(-(1-a)*y), y = x + b
                n = psum_ap.shape[-1]
                r = xp.tile([128, SC], F32, tag="lrtmp", name="lrtmp")
                rr = r[:, :n] if n != SC else r
                bias2 = 0.0
                if bias is not None:
                    key = (bias.tensor.name, bias.offset)
                    if key not in b2_cache:
                        b2 = wp.tile([128, 1], F32, name=f"b2_{len(b2_cache)}")
                        nc.vector.tensor_scalar_mul(out=b2, in0=bias,
                                                    scalar1=-(1.0 - ALPHA))
                        b2_cache[key] = b2
                    bias2 = b2_cache[key]
                nc.scalar.activation(out=rr, in_=psum_ap, func=RELU,
                                     bias=bias2, scale=-(1.0 - ALPHA))
                nc.vector.scalar_tensor_tensor(
                    out=out_ap, in0=psum_ap,
                    scalar=bias if bias is not None else 0.0,
                    op0=ADD, in1=rr, op1=ADD)

            for sc in range(n_sc):
                sc0 = sc * SC
                sa_enc = scp.tile([128, A, 2, SC], BF16, tag="sa_enc", name="sa_enc")
                s_enc = scp.tile([128, A, 2, SC], BF16, tag="s_enc", name="s_enc",
                                 bufs=2)
                # keys/sels stored feature-major with cols (chunk, b2, j) so the
                # s1 matmul operands are contiguous 128-col slices
                keys_fm = [scp.tile([128, SC * A], BF16, tag=f"keys{t}", name=f"keys{t}") for t in range(2)]
                sels_fm = [scp.tile([128, SC * A], BF16, tag=f"sels{t}", name=f"sels{t}") for t in range(2)]
                otherT = [scp.tile([128, A * SC], BF16, tag=f"oT{t}", name=f"oT{t}") for t in range(2)]
                allq_bm = scp.tile([128, NBT * 128], F32, tag="allq_bm", name="allq_bm")

                # ---------- encoder + keys/sels (feature-major) ----------
                for a in range(A):
                    x = xp.tile([128, 4, SC], BF16, tag="x", name="x")
                    nc.sync.dma_start(out=x, in_=saT_d[a, sc])
                    for mt in range(4):
                        ps = pmm.tile([128, SC], F32, tag="mm", name="mm")
                        for kt in range(4):
                            nc.tensor.matmul(out=ps,
                                             lhsT=w_enc_sb[a][:, kt, mt * 128:(mt + 1) * 128],
                                             rhs=x[:, kt, :],
                                             start=(kt == 0), stop=(kt == 3))
                        dst = sa_enc[:, a, mt, :] if mt < 2 else s_enc[:, a, mt - 2, :]
                        evict_lrelu(ps, dst, b_enc_sb[a][:, mt, :])
                    # keys (from sa_enc) and sels (from s_enc), feature-major,
                    # evicted into (chunk, j=a, b2) interleaved column order
                    # (16-wide contiguous runs; attention row rank = j*16+b2)
                    def ks_dst(t, a):
                        return bass.AP(tensor=t.tensor, offset=t.offset + a * 16,
                                       ap=[t.ap[0], [128, SC // 16], [1, 16]])
                    for et in range(2):
                        ps = pmm.tile([128, SC], F32, tag="mm", name="mm")
                        for kt in range(2):
                            nc.tensor.matmul(out=ps,
                                             lhsT=w_ks_sb[:, kt, et * 128:(et + 1) * 128],
                                             rhs=sa_enc[:, a, kt, :],
                                             start=(kt == 0), stop=(kt == 1))
                        nc.vector.tensor_copy(out=ks_dst(keys_fm[et], a), in_=ps)
                        ps = pmm.tile([128, SC], F32, tag="mm", name="mm")
                        for kt in range(2):
                            nc.tensor.matmul(out=ps,
                                             lhsT=w_ks_sb[:, kt, H + et * 128:H + (et + 1) * 128],
                                             rhs=s_enc[:, a, kt, :],
                                             start=(kt == 0), stop=(kt == 1))
                        nc.vector.tensor_copy(out=ks_dst(sels_fm[et], a), in_=ps)

                # ---------- per b-tile: vals, V2 bounce, s1, softmax, s2 ----------
                for bt in range(NBT if stages >= 2 else 0):
                    bt0 = bt * 128
                    # vals b-major [128b, (j, ed)]
                    vals_bm = btp.tile([128, A, ED], BF16, tag="vb", name="vb", bufs=1)
                    for a in range(A):
                        ps = pmm.tile([128, SC], F32, tag="mm", name="mm")
                        for kt in range(2):
                            nc.tensor.matmul(out=ps[:, 0:ED],
                                             lhsT=sa_enc[:, a, kt, bt0:bt0 + 128],
                                             rhs=w_val_sb[:, kt, :],
                                             start=(kt == 0), stop=False)
                        nc.tensor.matmul(out=ps[:, 0:ED], lhsT=ones_sb,
                                         rhs=bv2_sb[:, 0:ED], start=False, stop=True)
                        evict_lrelu(ps[:, 0:ED], vals_bm[:, a, :], None)
                    # V2 via DRAM bounce: vdram [c][j][b0][ed]; row rank = j*16+b0
                    vdram = dp.tile([8, A, 16, ED], BF16, tag="vd", name="vd")
                    for j in range(A):
                        nc.sync.dma_start(
                            out=bass.AP(tensor=vdram.tensor, offset=vdram.offset + j * 16 * ED,
                                        ap=[[A * 16 * ED, 8], [ED, 16], [1, ED]]),
                            in_=vals_bm[:, j, :])
                    V2C = 4 * 65  # 260 cols per chunk: 4 x (64 vals + ones col)
                    v2 = btp.tile([128, 8, V2C], BF16, tag="v2", name="v2")
                    for c in range(8):
                        src = bass.AP(tensor=vdram.tensor, offset=vdram.offset + c * A * 16 * ED,
                                      ap=[[ED, 128], [D, 4], [1, D]])
                        dst = bass.AP(tensor=v2.tensor, offset=v2.offset + c * V2C,
                                      ap=[v2.ap[0], [65, 4], [1, D]])
                        nc.gpsimd.dma_start(out=dst, in_=src)
                    # ones columns at e*65+64
                    nc.vector.memset(
                        bass.AP(tensor=v2.tensor, offset=v2.offset + D,
                                ap=[v2.ap[0], [V2C, 8], [65, 4]]), 1.0)

                    if stages < 3:
                        continue
                    othBM = btp.tile([128, 8, 4 * 65], BF16, tag="ob", name="ob",
                                     bufs=1)
                    for c in range(8):
                        # s1: logits c2-block per head.  Base-64 operands
                        # (odd heads) must target a psum tile at bank offset 0,
                        # so e1/e3 get their own tiles.
                        cg = bt * 8 + c  # chunk index within superchunk
                        pev = ps1.tile([128, 256], F32, tag="s1e", name="s1e")
                        pod = [ps1.tile([128, 128], F32, tag=f"s1o{k}", name=f"s1o{k}")
                               for k in range(2)]
                        for e in range(4):
                            et, eh = e // 2, (e % 2) * 64
                            lhsT = _ap64(keys_fm[et], eh, cg * 128, SC)
                            rhs = _ap64(sels_fm[et], eh, cg * 128, SC)
                            out = (pev[:, (e // 2) * 128:(e // 2 + 1) * 128]
                                   if e % 2 == 0 else pod[e // 2])
                            nc.tensor.matmul(out=out, lhsT=lhsT, rhs=rhs,
                                             start=True, stop=True)
                        # exp (scale 1/sqrt(D)); pu cols stay in (e, b1, i) order
                        pu = pup.tile([128, 512], BF16, tag="pu", name="pu")
                        escale = 1.0 / np.sqrt(np.float32(D))
                        nc.scalar.activation(
                            out=bass.AP(tensor=pu.tensor, offset=pu.offset,
                                        ap=[pu.ap[0], [256, 2], [1, 128]]),
                            in_=pev, func=EXP, bias=0.0, scale=escale)
                        for k in range(2):
                            nc.scalar.activation(out=pu[:, (2 * k + 1) * 128:(2 * k + 2) * 128],
                                                 in_=pod[k], func=EXP, bias=0.0,
                                                 scale=escale)
                        # mask: zero off-diagonal blocks and self-agent
                        pm = pup.tile([128, 512], BF16, tag="pm", name="pm")
                        nc.vector.tensor_tensor(
                            out=pm,
                            in0=pu,
                            in1=bass.AP(tensor=dmask_sb.tensor, offset=dmask_sb.offset,
                                        ap=[dmask_sb.ap[0], [0, 4], [1, 128]]),
                            op=MULT)
                        if stages < 4:
                            continue
                        # s2: other_bm + Z column per head
                        p2 = ps2.tile([128, 4 * 65], F32, tag="s2", name="s2")
                        for e in range(4):
                            nc.tensor.matmul(out=p2[:, e * 65:(e + 1) * 65],
                                             lhsT=pm[:, e * 128:(e + 1) * 128],
                                             rhs=v2[:, c, e * 65:(e + 1) * 65],
                                             start=True, stop=True)
                        nc.vector.tensor_copy(out=othBM[:, c, :], in_=p2)

                    if stages < 4:
                        continue
                    # normalize: rZ = 1/Z, othN = othBM * rZ (broadcast over d)
                    rz = btp.tile([128, 8, 4], BF16, tag="rz", name="rz")
                    with nc.allow_low_precision(reason="1/Z in bf16 is within tolerance"):
                        nc.vector.reciprocal(
                            out=rz,
                            in_=bass.AP(tensor=othBM.tensor, offset=othBM.offset + D,
                                        ap=[othBM.ap[0], [4 * 65, 8], [65, 4]]))
                    othN = btp.tile([128, 8 * 4 * D], BF16, tag="on", name="on")
                    nc.vector.tensor_tensor(
                        out=othN,
                        in0=bass.AP(tensor=othBM.tensor, offset=othBM.offset,
                                    ap=[othBM.ap[0], [4 * 65, 8], [65, 4], [1, D]]),
                        in1=bass.AP(tensor=rz.tensor, offset=rz.offset,
                                    ap=[rz.ap[0], [4, 8], [1, 4], [0, D]]),
                        op=MULT)

                    # transpose back to feature-major: otherT[ep][:, (i, bt, c, b1)]
                    # one [128,128] transpose covers both heads of an e-pair
                    for ch2 in range(2):
                        for ep in range(2):
                            pt = ptp.tile([128, 512], BF16, tag="tr", name="tr")
                            for c2 in range(4):
                                cg = ch2 * 4 + c2
                                nc.tensor.transpose(
                                    out=pt[:, c2 * 128:(c2 + 1) * 128],
                                    in_=othN[:, (cg * 4 + ep * 2) * D:(cg * 4 + ep * 2 + 2) * D],
                                    identity=identb)
                            # evict to otherT[ep] cols (i*SC + bt*128 + c*16 + b1);
                            # pt cols decode as (c2, i, b1) with the (i*16+b1) rank
                            dst = bass.AP(
                                tensor=otherT[ep].tensor,
                                offset=otherT[ep].offset + bt0 + ch2 * 64,
                                ap=[otherT[ep].ap[0], [16, 4], [SC, 8], [1, 16]])
                            nc.vector.tensor_copy(out=dst, in_=pt)

                # ---------- critic (per agent over the whole superchunk) ----------
                if stages < 5:
                    nc.vector.memset(allq_bm, 0.0)
                for a in range(A if stages >= 5 else 0):
                    h_t = hp.tile([128, 2, SC], BF16, tag="h", name="h")
                    for mt in range(2):
                        ps = pmm.tile([128, SC], F32, tag="mm", name="mm")
                        for kt in range(2):
                            nc.tensor.matmul(out=ps,
                                             lhsT=w_c1_sb[a][:, kt, mt * 128:(mt + 1) * 128],
                                             rhs=s_enc[:, a, kt, :],
                                             start=(kt == 0), stop=False)
                        for kt in range(2):
                            nc.tensor.matmul(out=ps,
                                             lhsT=w_c1_sb[a][:, 2 + kt, mt * 128:(mt + 1) * 128],
                                             rhs=otherT[kt][:, a * SC:(a + 1) * SC],
                                             start=False, stop=(kt == 1))
                        evict_lrelu(ps, h_t[:, mt, :], b_c1_sb[a][:, mt, :])
                    allq_a = hp.tile([16, SC], F32, tag="aq", name="aq", bufs=1)
                    psq = pmm.tile([128, SC], F32, tag="mm", name="mm")
                    for kt in range(2):
                        nc.tensor.matmul(out=psq[0:NACT, :], lhsT=w_c2_sb[a][:, kt, :],
                                         rhs=h_t[:, kt, :],
                                         start=(kt == 0), stop=(kt == 1))
                    nc.scalar.activation(out=allq_a, in_=psq[0:NACT, :], func=IDENT,
                                         bias=b_c2_sb[a], scale=1.0)
                    ptq = pmm.tile([128, SC], F32, tag="mm", name="mm")
                    for bt in range(NBT):
                        nc.tensor.transpose(out=ptq[:, bt * 16:(bt + 1) * 16],
                                            in_=allq_a[:, bt * 128:(bt + 1) * 128],
                                            identity=ident16)
                    # allq_bm cols (bt, a, o)
                    nc.vector.tensor_copy(
                        out=bass.AP(tensor=allq_bm.tensor,
                                    offset=allq_bm.offset + a * NACT,
                                    ap=[allq_bm.ap[0], [128, NBT], [1, NACT]]),
                        in_=ptq[:, 0:NBT * 16])

                # ---------- argmax gather (per b-tile) ----------
                for bt in range(NBT):
                    b0 = sc0 + bt * 128
                    acts_t = btp.tile([128, A, NACT], F32, tag="at", name="at")
                    nc.sync.dma_start(out=acts_t, in_=acts_d[b0:b0 + 128])
                    amax = btp.tile([128, A], F32, tag="am", name="am")
                    nc.vector.tensor_reduce(out=amax, in_=acts_t, axis=X, op=MAX)
                    oh1 = btp.tile([128, A, NACT], F32, tag="oh1", name="oh1", bufs=1)
                    nc.vector.tensor_tensor(
                        out=oh1.rearrange("p a o -> p (a o)"),
                        in0=acts_t.rearrange("p a o -> p (a o)"),
                        in1=bass.AP(tensor=amax.tensor, offset=amax.offset,
                                    ap=[amax.ap[0], [1, A], [0, NACT]]),
                        op=ISGE)
                    # tie-break: keep only the first (lowest-index) max
                    val = btp.tile([128, A, NACT], F32, tag="val", name="val", bufs=1)
                    nc.vector.tensor_tensor(out=val.rearrange("p a o -> p (a o)"),
                                            in0=oh1.rearrange("p a o -> p (a o)"),
                                            in1=w16_sb, op=MULT)
                    m2 = btp.tile([128, A], F32, tag="m2", name="m2")
                    nc.vector.tensor_reduce(out=m2, in_=val, axis=X, op=MAX)
                    oh2 = btp.tile([128, A, NACT], F32, tag="oh2", name="oh2", bufs=1)
                    nc.vector.tensor_tensor(
                        out=oh2.rearrange("p a o -> p (a o)"),
                        in0=val.rearrange("p a o -> p (a o)"),
                        in1=bass.AP(tensor=m2.tensor, offset=m2.offset,
                                    ap=[m2.ap[0], [1, A], [0, NACT]]),
                        op=ISGE)
                    qm = btp.tile([128, A, NACT], F32, tag="qm", name="qm", bufs=1)
                    nc.vector.tensor_tensor(
                        out=qm.rearrange("p a o -> p (a o)"),
                        in0=oh2.rearrange("p a o -> p (a o)"),
                        in1=allq_bm[:, bt * 128:(bt + 1) * 128],
                        op=MULT)
                    q_sb = btp.tile([128, A], F32, tag="qs", name="qs")
                    nc.vector.tensor_reduce(out=q_sb, in_=qm, axis=X, op=ADD)
                    nc.sync.dma_start(out=q_d[b0:b0 + 128], in_=q_sb)

    nc.compile()
    return nc


def _ap64(t, eh, col, SC):
    """64 partitions at base eh; 128 contiguous cols at `col`."""
    row = A * SC  # free elements per partition of the [128, SC*A] tile
    return bass.AP(tensor=t.tensor, offset=t.offset + eh * row + col,
                   ap=[[row, 64], [1, 128]])


def _prep_inputs(states, actions, enc_W, enc_b, s_W, s_b, key_W, sel_W,
                 val_W, val_b, c_W1, c_b1, c_W2, c_b2,
                 b_local=B_LOCAL, n_cores=N_CORES):
    f32 = np.float32
    Bv = b_local * n_cores
    states = states[:, :Bv]
    actions = actions[:, :Bv]
    sa = np.concatenate([states, actions], axis=-1).astype(f32)
    saT = np.zeros((A, KPAD, Bv), dtype=bfloat16)
    saT[:, :IDIM, :] = sa.transpose(0, 2, 1).astype(bfloat16)
    SCW = min(512, b_local)
    n_sc = b_local // SCW
    w_enc = np.zeros((A, KPAD, 2 * H), dtype=bfloat16)
    w_enc[:, :IDIM, :H] = enc_W.astype(bfloat16)
    w_enc[:, :S, H:] = s_W.astype(bfloat16)
    b_enc = np.concatenate([enc_b, s_b], axis=-1).astype(f32).reshape(A, 4, 128, 1)
    w_ks = np.zeros((H, 2 * H), dtype=bfloat16)
    w_ks[:, 0:H] = key_W.transpose(1, 0, 2).reshape(H, H).astype(bfloat16)
    w_ks[:, H:] = sel_W.transpose(1, 0, 2).reshape(H, H).astype(bfloat16)
    w_val = val_W.transpose(1, 0, 2).reshape(H, H).astype(bfloat16)
    bv = np.tile(val_b.reshape(1, -1), (1, 2)).astype(bfloat16)  # doubled: 2 agents/psum
    w_c1 = c_W1.astype(bfloat16)
    b_c1 = c_b1.astype(f32).reshape(A, 2, 128, 1)
    w_c2 = c_W2.astype(bfloat16)
    b_c2 = c_b2.astype(f32).reshape(A, NACT, 1)
    # dmask[(j*16+b2), (i*16+b1)] = (b2==b1) & (i != j)
    dmask = np.zeros((128, 128), dtype=bfloat16)
    for b2 in range(16):
        for j in range(A):
            for i in range(A):
                if i != j:
                    dmask[j * 16 + b2, i * 16 + b2] = 1
    w16 = np.broadcast_to(
        (NACT - np.arange(NACT, dtype=f32))[None, None, :],
        (128, A, NACT)).reshape(128, A * NACT).copy()
    acts_bm = actions.transpose(1, 0, 2).astype(f32)

    shared = dict(w_enc=w_enc, b_enc=b_enc, w_ks=w_ks, w_val=w_val, bv=bv,
                  w_c1=w_c1, b_c1=b_c1, w_c2=w_c2, b_c2=b_c2,
                  dmask=dmask, w16=w16)
    in_maps = []
    for cid in range(n_cores):
        sl = slice(cid * b_local, (cid + 1) * b_local)
        m_ = dict(shared)
        # pack [A, KPAD, b_local] -> [A, n_sc, 128, 4, SCW] so each partition's
        # x-load is one contiguous 4KB run
        sc_ = saT[:, :, sl].reshape(A, 4, 128, n_sc, SCW)
        m_["saT"] = np.ascontiguousarray(sc_.transpose(0, 3, 2, 1, 4))
        m_["acts"] = np.ascontiguousarray(acts_bm[sl])
        in_maps.append(m_)
    return in_maps


_NC_CACHE = {}


def _get_nc(b_local=B_LOCAL):
    if b_local not in _NC_CACHE:
        _NC_CACHE[b_local] = build_bass(b_local)
    return _NC_CACHE[b_local]


def kernel(**inputs):
    inputs = {k: np.asarray(v) for k, v in inputs.items()}
    in_maps = _prep_inputs(**inputs)
    nc = _get_nc()
    res = bass_utils.run_bass_kernel_spmd(
        nc, in_maps, core_ids=list(range(N_CORES)),
        trace=bool(int(os.environ.get("MAAC_TRACE", "0"))))
    q = np.concatenate([r["q"].T for r in res.results], axis=1)  # [A, B]
    if res.exec_time_ns is not None:
        print(f"HW exec time: {res.exec_time_ns} ns")
    return q[:, :, None].astype(np.float32)


# revision 5
# speedup vs baseline: 1.0184x; 1.0033x over previous
"""MAAC critic kernel v2 for Trainium2 — attention on the TensorEngine.

Data-parallel over batch on 8 cores. Per core (b_local=2048), processed in
superchunks of SC=512 (b-tiles of 128, chunks of 16):

  encoder/kvs: feature-major weights-stationary matmuls (bf16).
  stage-1 logits: per (chunk16, head) ONE PE matmul
      out[(b2*8+j), (b1*8+i)] = sum_d keys[d,(b2,j)] * sels[d,(b1,i)]
    — the b2==b1 diagonal blocks are the real logits; the rest is cheap waste.
  softmax stays in that domain: exp on the whole block (ScalarE), then one
    DVE multiply with a constant mask [(b2 j),(b1 i)] = (b2==b1)&(i!=j)
    which zeroes both the off-diagonal garbage and the self-agent term.
  stage-2: out[(b1 i), (d|Z)] = Pm^T @ [V2 | ones] — block-diag structure of
    Pm makes this exact; the appended ones-column yields the softmax
    denominator Z per (b, i) for free.  V2 = vals in [(b0*8+j), ed] layout
    via a dense DRAM round trip (the only partition reshuffle).
  normalize: DVE reciprocal of the Z columns + one broadcast multiply.
  transpose attended values back feature-major via PE transposes, then
  per-agent critic MLP, PE-transpose of q, fp32 one-hot gather with a
  first-index tie-break matching jnp.argmax.
"""
import os
import numpy as np
import ml_dtypes

import concourse.bass as bass
import concourse.tile as tile
import concourse.mybir as mybir
from concourse import bacc
from concourse import bass_utils
from concourse.masks import make_identity

F32 = mybir.dt.float32
BF16 = mybir.dt.bfloat16
bfloat16 = ml_dtypes.bfloat16

A = 8
B = 16384
S = 456
NACT = 16
IDIM = S + NACT
KPAD = 512
H = 256
E = 4
D = 64
ED = E * D               # 256
N_CORES = 8
B_LOCAL = B // N_CORES   # 2048
ALPHA = 0.01


def build_bass(b_local=B_LOCAL, lrelu_act=True, stages=5):
    """stages: 1=enc/kvs, 2=+vals/V2, 3=+s1/softmax, 4=+s2/norm/transpose, 5=all."""
    nc = bacc.Bacc("TRN2", target_bir_lowering=False, debug=False)
    SC = min(512, b_local)       # superchunk width (free dim of big matmuls)
    n_sc = b_local // SC
    NBT = SC // 128              # b-tiles per superchunk
    LRELU = mybir.ActivationFunctionType.Lrelu
    EXP = mybir.ActivationFunctionType.Exp
    IDENT = mybir.ActivationFunctionType.Identity
    MULT = mybir.AluOpType.mult
    ADD = mybir.AluOpType.add
    ISGE = mybir.AluOpType.is_ge
    MAX = mybir.AluOpType.max
    X = mybir.AxisListType.X

    # ---- DRAM I/O ----
    saT_d = nc.dram_tensor("saT", [A, b_local // min(512, b_local), 128, 4,
                                   min(512, b_local)], BF16, kind="ExternalInput")
    acts_d = nc.dram_tensor("acts", [b_local, A, NACT], F32, kind="ExternalInput")
    w_enc_d = nc.dram_tensor("w_enc", [A, KPAD, 2 * H], BF16, kind="ExternalInput")
    b_enc_d = nc.dram_tensor("b_enc", [A, 4, 128, 1], F32, kind="ExternalInput")
    w_ks_d = nc.dram_tensor("w_ks", [H, 2 * H], BF16, kind="ExternalInput")
    w_val_d = nc.dram_tensor("w_val", [H, ED], BF16, kind="ExternalInput")
    bv_d = nc.dram_tensor("bv", [1, 2 * ED], BF16, kind="ExternalInput")
    w_c1_d = nc.dram_tensor("w_c1", [A, 2 * H, H], BF16, kind="ExternalInput")
    b_c1_d = nc.dram_tensor("b_c1", [A, 2, 128, 1], F32, kind="ExternalInput")
    w_c2_d = nc.dram_tensor("w_c2", [A, H, NACT], BF16, kind="ExternalInput")
    b_c2_d = nc.dram_tensor("b_c2", [A, NACT, 1], F32, kind="ExternalInput")
    dmask_d = nc.dram_tensor("dmask", [128, 128], BF16, kind="ExternalInput")
    w16_d = nc.dram_tensor("w16", [128, A * NACT], F32, kind="ExternalInput")
    q_d = nc.dram_tensor("q", [b_local, A], F32, kind="ExternalOutput")

    with tile.TileContext(nc) as tc:
        import contextlib
        with contextlib.ExitStack() as ctx:
            wp = ctx.enter_context(tc.tile_pool(name="wp", bufs=1))
            xp = ctx.enter_context(tc.tile_pool(name="xp", bufs=2))
            scp = ctx.enter_context(tc.tile_pool(name="scp", bufs=1))
            btp = ctx.enter_context(tc.tile_pool(name="btp", bufs=2))
            pup = ctx.enter_context(tc.tile_pool(name="pup", bufs=2))
            hp = ctx.enter_context(tc.tile_pool(name="hp", bufs=2))
            pmm = ctx.enter_context(tc.tile_pool(name="pmm", bufs=3, space="PSUM"))
            ps1 = ctx.enter_context(tc.tile_pool(name="ps1", bufs=1, space="PSUM"))
            ps2 = ctx.enter_context(tc.tile_pool(name="ps2", bufs=1, space="PSUM"))
            ptp = ctx.enter_context(tc.tile_pool(name="ptp", bufs=1, space="PSUM"))
            dp = ctx.enter_context(tc.tile_pool(name="dp", bufs=2, space="DRAM"))

            # ---- resident weights ----
            w_enc_sb, b_enc_sb = [], []
            w_c1_sb, b_c1_sb, w_c2_sb, b_c2_sb = [], [], [], []
            for a in range(A):
                w = wp.tile([128, 4, 2 * H], BF16, name=f"w_enc{a}")
                (nc.sync if a == 0 else nc.scalar).dma_start(
                    out=w, in_=w_enc_d[a].rearrange("(kt p) m -> p kt m", p=128))
                w_enc_sb.append(w)
                bt_ = wp.tile([128, 4, 1], F32, name=f"b_enc{a}")
                nc.sync.dma_start(out=bt_, in_=b_enc_d[a].rearrange("kt p one -> p kt one"))
                b_enc_sb.append(bt_)
                w1 = wp.tile([128, 4, H], BF16, name=f"w_c1{a}")
                nc.scalar.dma_start(out=w1, in_=w_c1_d[a].rearrange("(kt p) m -> p kt m", p=128))
                w_c1_sb.append(w1)
                b1 = wp.tile([128, 2, 1], F32, name=f"b_c1{a}")
                nc.scalar.dma_start(out=b1, in_=b_c1_d[a].rearrange("kt p one -> p kt one"))
                b_c1_sb.append(b1)
                w2 = wp.tile([128, 2, NACT], BF16, name=f"w_c2{a}")
                nc.scalar.dma_start(out=w2, in_=w_c2_d[a].rearrange("(kt p) m -> p kt m", p=128))
                w_c2_sb.append(w2)
                b2 = wp.tile([NACT, 1], F32, name=f"b_c2{a}")
                nc.scalar.dma_start(out=b2, in_=b_c2_d[a])
                b_c2_sb.append(b2)
            w_ks_sb = wp.tile([128, 2, 2 * H], BF16, name="w_ks")
            nc.sync.dma_start(out=w_ks_sb, in_=w_ks_d.rearrange("(kt p) m -> p kt m", p=128))
            w_val_sb = wp.tile([128, 2, ED], BF16, name="w_val")
            nc.sync.dma_start(out=w_val_sb, in_=w_val_d.rearrange("(kt p) m -> p kt m", p=128))
            bv2_sb = wp.tile([1, 2 * ED], BF16, name="bv2")
            nc.sync.dma_start(out=bv2_sb, in_=bv_d[:, :])
            ones_sb = wp.tile([1, 128], BF16, name="ones")
            nc.vector.memset(ones_sb, 1.0)
            dmask_sb = wp.tile([128, 128], BF16, name="dmask")
            nc.sync.dma_start(out=dmask_sb, in_=dmask_d[:, :])
            w16_sb = wp.tile([128, A * NACT], F32, name="w16")
            nc.sync.dma_start(out=w16_sb, in_=w16_d[:, :])
            identb = wp.tile([128, 128], BF16, name="identb")
            make_identity(nc, identb)
            ident16 = wp.tile([16, 16], F32, name="ident16")
            make_identity(nc, ident16)

            RELU = mybir.ActivationFunctionType.Relu
            b2_cache = {}

            def evict_lrelu(psum_ap, out_ap, bias, eng=None):
                """out = lrelu(psum + bias); bias is a [128,1] AP or None.

                Table-free on ScalarE (Identity w/ bias) so the Exp act-table
                stays resident; the leak is one max(y, alpha*y) on eng."""
                if lrelu_act:
                    n = psum_ap.shape[-1]
                    tmp = pup.tile([128, SC], BF16, tag="lr", name="lr", bufs=3)
                    tn = tmp[:, :n] if n != SC else tmp
                    nc.scalar.activation(out=tn, in_=psum_ap, func=IDENT,
                                         bias=bias if bias is not None else 0.0,
                                         scale=1.0)
                    (eng or nc.vector).scalar_tensor_tensor(
                        out=out_ap, in0=tn, scalar=ALPHA, op0=MULT,
                        in1=tn, op1=mybir.AluOpType.max)
                    return
                # CoreSim fallback: lrelu(y) = y + relu(-(1-a)*y), y = x + b
                n = psum_ap.shape[-1]
                r = xp.tile([128, SC], F32, tag="lrtmp", name="lrtmp")
                rr = r[:, :n] if n != SC else r
                bias2 = 0.0
                if bias is not None:
                    key = (bias.tensor.name, bias.offset)
                    if key not in b2_cache:
                        b2 = wp.tile([128, 1], F32, name=f"b2_{len(b2_cache)}")
                        nc.vector.tensor_scalar_mul(out=b2, in0=bias,
                                                    scalar1=-(1.0 - ALPHA))
                        b2_cache[key] = b2
                    bias2 = b2_cache[key]
                nc.scalar.activation(out=rr, in_=psum_ap, func=RELU,
                                     bias=bias2, scale=-(1.0 - ALPHA))
                nc.vector.scalar_tensor_tensor(
                    out=out_ap, in0=psum_ap,
                    scalar=bias if bias is not None else 0.0,
                    op0=ADD, in1=rr, op1=ADD)

            for sc in range(n_sc):
                sc0 = sc * SC
                sa_enc = scp.tile([128, A, 2, SC], BF16, tag="sa_enc", name="sa_enc")
                s_enc = scp.tile([128, A, 2, SC], BF16, tag="s_enc", name="s_enc",
                                 bufs=2)
                # keys/sels stored feature-major with cols (chunk, b2, j) so the
                # s1 matmul operands are contiguous 128-col slices
                keys_fm = [scp.tile([128, SC * A], BF16, tag=f"keys{t}", name=f"keys{t}") for t in range(2)]
                sels_fm = [scp.tile([128, SC * A], BF16, tag=f"sels{t}", name=f"sels{t}") for t in range(2)]
                otherT = [scp.tile([128, A * SC], BF16, tag=f"oT{t}", name=f"oT{t}") for t in range(2)]
                allq_bm = scp.tile([128, NBT * 128], F32, tag="allq_bm", name="allq_bm")

                # ---------- encoder + keys/sels (feature-major) ----------
                for a in range(A):
                    x = xp.tile([128, 4, SC], BF16, tag="x", name="x")
                    nc.sync.dma_start(out=x, in_=saT_d[a, sc])
                    for mt in range(4):
                        ps = pmm.tile([128, SC], F32, tag="mm", name="mm")
                        for kt in range(4):
                            nc.tensor.matmul(out=ps,
                                             lhsT=w_enc_sb[a][:, kt, mt * 128:(mt + 1) * 128],
                                             rhs=x[:, kt, :],
                                             start=(kt == 0), stop=(kt == 3))
                        dst = sa_enc[:, a, mt, :] if mt < 2 else s_enc[:, a, mt - 2, :]
                        evict_lrelu(ps, dst, b_enc_sb[a][:, mt, :])
                    # keys (from sa_enc) and sels (from s_enc), feature-major,
                    # evicted into (chunk, j=a, b2) interleaved column order
                    # (16-wide contiguous runs; attention row rank = j*16+b2)
                    def ks_dst(t, a):
                        return bass.AP(tensor=t.tensor, offset=t.offset + a * 16,
                                       ap=[t.ap[0], [128, SC // 16], [1, 16]])
                    for et in range(2):
                        ps = pmm.tile([128, SC], F32, tag="mm", name="mm")
                        for kt in range(2):
                            nc.tensor.matmul(out=ps,
                                             lhsT=w_ks_sb[:, kt, et * 128:(et + 1) * 128],
                                             rhs=sa_enc[:, a, kt, :],
                                             start=(kt == 0), stop=(kt == 1))
                        nc.vector.tensor_copy(out=ks_dst(keys_fm[et], a), in_=ps)
                        ps = pmm.tile([128, SC], F32, tag="mm", name="mm")
                        for kt in range(2):
                            nc.tensor.matmul(out=ps,
                                             lhsT=w_ks_sb[:, kt, H + et * 128:H + (et + 1) * 128],
                                             rhs=s_enc[:, a, kt, :],
                                             start=(kt == 0), stop=(kt == 1))
                        nc.vector.tensor_copy(out=ks_dst(sels_fm[et], a), in_=ps)

                # ---------- per b-tile: vals, V2 bounce, s1, softmax, s2 ----------
                for bt in range(NBT if stages >= 2 else 0):
                    bt0 = bt * 128
                    # vals b-major [128b, (j, ed)]
                    vals_bm = btp.tile([128, A, ED], BF16, tag="vb", name="vb", bufs=1)
                    for a in range(A):
                        ps = pmm.tile([128, SC], F32, tag="mm", name="mm")
                        for kt in range(2):
                            nc.tensor.matmul(out=ps[:, 0:ED],
                                             lhsT=sa_enc[:, a, kt, bt0:bt0 + 128],
                                             rhs=w_val_sb[:, kt, :],
                                             start=(kt == 0), stop=False)
                        nc.tensor.matmul(out=ps[:, 0:ED], lhsT=ones_sb,
                                         rhs=bv2_sb[:, 0:ED], start=False, stop=True)
                        evict_lrelu(ps[:, 0:ED], vals_bm[:, a, :], None)
                    # V2 via DRAM bounce: vdram [c][j][b0][ed]; row rank = j*16+b0
                    vdram = dp.tile([8, A, 16, ED], BF16, tag="vd", name="vd")
                    for j in range(A):
                        nc.sync.dma_start(
                            out=bass.AP(tensor=vdram.tensor, offset=vdram.offset + j * 16 * ED,
                                        ap=[[A * 16 * ED, 8], [ED, 16], [1, ED]]),
                            in_=vals_bm[:, j, :])
                    V2C = 4 * 65  # 260 cols per chunk: 4 x (64 vals + ones col)
                    v2 = btp.tile([128, 8, V2C], BF16, tag="v2", name="v2")
                    for c in range(8):
                        src = bass.AP(tensor=vdram.tensor, offset=vdram.offset + c * A * 16 * ED,
                                      ap=[[ED, 128], [D, 4], [1, D]])
                        dst = bass.AP(tensor=v2.tensor, offset=v2.offset + c * V2C,
                                      ap=[v2.ap[0], [65, 4], [1, D]])
                        nc.gpsimd.dma_start(out=dst, in_=src)
                    # ones columns at e*65+64
                    nc.vector.memset(
                        bass.AP(tensor=v2.tensor, offset=v2.offset + D,
                                ap=[v2.ap[0], [V2C, 8], [65, 4]]), 1.0)

                    if stages < 3:
                        continue
                    othBM = btp.tile([128, 8, 4 * 65], BF16, tag="ob", name="ob",
                                     bufs=1)
                    for c in range(8):
                        # s1: logits c2-block per head.  Base-64 operands
                        # (odd heads) must target a psum tile at bank offset 0,
                        # so e1/e3 get their own tiles.
                        cg = bt * 8 + c  # chunk index within superchunk
                        pev = ps1.tile([128, 256], F32, tag="s1e", name="s1e")
                        pod = [ps1.tile([128, 128], F32, tag=f"s1o{k}", name=f"s1o{k}")
                               for k in range(2)]
                        for e in range(4):
                            et, eh = e // 2, (e % 2) * 64
                            lhsT = _ap64(keys_fm[et], eh, cg * 128, SC)
                            rhs = _ap64(sels_fm[et], eh, cg * 128, SC)
                            out = (pev[:, (e // 2) * 128:(e // 2 + 1) * 128]
                                   if e % 2 == 0 else pod[e // 2])
                            nc.tensor.matmul(out=out, lhsT=lhsT, rhs=rhs,
                                             start=True, stop=True)
                        # exp (scale 1/sqrt(D)); pu cols stay in (e, b1, i) order
                        pu = pup.tile([128, 512], BF16, tag="pu", name="pu")
                        escale = 1.0 / np.sqrt(np.float32(D))
                        nc.scalar.activation(
                            out=bass.AP(tensor=pu.tensor, offset=pu.offset,
                                        ap=[pu.ap[0], [256, 2], [1, 128]]),
                            in_=pev, func=EXP, bias=0.0, scale=escale)
                        for k in range(2):
                            nc.scalar.activation(out=pu[:, (2 * k + 1) * 128:(2 * k + 2) * 128],
                                                 in_=pod[k], func=EXP, bias=0.0,
                                                 scale=escale)
                        # mask: zero off-diagonal blocks and self-agent
                        pm = pup.tile([128, 512], BF16, tag="pm", name="pm")
                        nc.vector.tensor_tensor(
                            out=pm,
                            in0=pu,
                            in1=bass.AP(tensor=dmask_sb.tensor, offset=dmask_sb.offset,
                                        ap=[dmask_sb.ap[0], [0, 4], [1, 128]]),
                            op=MULT)
                        if stages < 4:
                            continue
                        # s2: other_bm + Z column per head
                        p2 = ps2.tile([128, 4 * 65], F32, tag="s2", name="s2")
                        for e in range(4):
                            nc.tensor.matmul(out=p2[:, e * 65:(e + 1) * 65],
                                             lhsT=pm[:, e * 128:(e + 1) * 128],
                                             rhs=v2[:, c, e * 65:(e + 1) * 65],
                                             start=True, stop=True)
                        nc.vector.tensor_copy(out=othBM[:, c, :], in_=p2)

                    if stages < 4:
                        continue
                    # normalize: rZ = 1/Z, othN = othBM * rZ (broadcast over d)
                    rz = btp.tile([128, 8, 4], BF16, tag="rz", name="rz")
                    with nc.allow_low_precision(reason="1/Z in bf16 is within tolerance"):
                        nc.vector.reciprocal(
                            out=rz,
                            in_=bass.AP(tensor=othBM.tensor, offset=othBM.offset + D,
                                        ap=[othBM.ap[0], [4 * 65, 8], [65, 4]]))
                    othN = btp.tile([128, 8 * 4 * D], BF16, tag="on", name="on")
                    nc.vector.tensor_tensor(
                        out=othN,
                        in0=bass.AP(tensor=othBM.tensor, offset=othBM.offset,
                                    ap=[othBM.ap[0], [4 * 65, 8], [65, 4], [1, D]]),
                        in1=bass.AP(tensor=rz.tensor, offset=rz.offset,
                                    ap=[rz.ap[0], [4, 8], [1, 4], [0, D]]),
                        op=MULT)

                    # transpose back to feature-major: otherT[ep][:, (i, bt, c, b1)]
                    # one [128,128] transpose covers both heads of an e-pair
                    for ch2 in range(2):
                        for ep in range(2):
                            pt = ptp.tile([128, 512], BF16, tag="tr", name="tr")
                            for c2 in range(4):
                                cg = ch2 * 4 + c2
                                nc.tensor.transpose(
                                    out=pt[:, c2 * 128:(c2 + 1) * 128],
                                    in_=othN[:, (cg * 4 + ep * 2) * D:(cg * 4 + ep * 2 + 2) * D],
                                    identity=identb)
                            # evict to otherT[ep] cols (i*SC + bt*128 + c*16 + b1);
                            # pt cols decode as (c2, i, b1) with the (i*16+b1) rank
                            dst = bass.AP(
                                tensor=otherT[ep].tensor,
                                offset=otherT[ep].offset + bt0 + ch2 * 64,
                                ap=[otherT[ep].ap[0], [16, 4], [SC, 8], [1, 16]])
                            nc.vector.tensor_copy(out=dst, in_=pt)

                # ---------- critic (per agent over the whole superchunk) ----------
                if stages < 5:
                    nc.vector.memset(allq_bm, 0.0)
                for a in range(A if stages >= 5 else 0):
                    h_t = hp.tile([128, 2, SC], BF16, tag="h", name="h")
                    for mt in range(2):
                        ps = pmm.tile([128, SC], F32, tag="mm", name="mm")
                        for kt in range(2):
                            nc.tensor.matmul(out=ps,
                                             lhsT=w_c1_sb[a][:, kt, mt * 128:(mt + 1) * 128],
                                             rhs=s_enc[:, a, kt, :],
                                             start=(kt == 0), stop=False)
                        for kt in range(2):
                            nc.tensor.matmul(out=ps,
                                             lhsT=w_c1_sb[a][:, 2 + kt, mt * 128:(mt + 1) * 128],
                                             rhs=otherT[kt][:, a * SC:(a + 1) * SC],
                                             start=False, stop=(kt == 1))
                        evict_lrelu(ps, h_t[:, mt, :], b_c1_sb[a][:, mt, :])
                    allq_a = hp.tile([16, SC], F32, tag="aq", name="aq", bufs=1)
                    psq = pmm.tile([128, SC], F32, tag="mm", name="mm")
                    for kt in range(2):
                        nc.tensor.matmul(out=psq[0:NACT, :], lhsT=w_c2_sb[a][:, kt, :],
                                         rhs=h_t[:, kt, :],
                                         start=(kt == 0), stop=(kt == 1))
                    nc.scalar.activation(out=allq_a, in_=psq[0:NACT, :], func=IDENT,
                                         bias=b_c2_sb[a], scale=1.0)
                    ptq = pmm.tile([128, SC], F32, tag="mm", name="mm")
                    for bt in range(NBT):
                        nc.tensor.transpose(out=ptq[:, bt * 16:(bt + 1) * 16],
                                            in_=allq_a[:, bt * 128:(bt + 1) * 128],
                                            identity=ident16)
                    # allq_bm cols (bt, a, o)
                    nc.vector.tensor_copy(
                        out=bass.AP(tensor=allq_bm.tensor,
                                    offset=allq_bm.offset + a * NACT,
                                    ap=[allq_bm.ap[0], [128, NBT], [1, NACT]]),
                        in_=ptq[:, 0:NBT * 16])

                # ---------- argmax gather (per b-tile) ----------
                for bt in range(NBT):
                    b0 = sc0 + bt * 128
                    acts_t = btp.tile([128, A, NACT], F32, tag="at", name="at")
                    nc.sync.dma_start(out=acts_t, in_=acts_d[b0:b0 + 128])
                    amax = btp.tile([128, A], F32, tag="am", name="am")
                    nc.vector.tensor_reduce(out=amax, in_=acts_t, axis=X, op=MAX)
                    oh1 = btp.tile([128, A, NACT], F32, tag="oh1", name="oh1", bufs=1)
                    nc.vector.tensor_tensor(
                        out=oh1.rearrange("p a o -> p (a o)"),
                        in0=acts_t.rearrange("p a o -> p (a o)"),
                        in1=bass.AP(tensor=amax.tensor, offset=amax.offset,
                                    ap=[amax.ap[0], [1, A], [0, NACT]]),
                        op=ISGE)
                    # tie-break: keep only the first (lowest-index) max
                    val = btp.tile([128, A, NACT], F32, tag="val", name="val", bufs=1)
                    nc.vector.tensor_tensor(out=val.rearrange("p a o -> p (a o)"),
                                            in0=oh1.rearrange("p a o -> p (a o)"),
                                            in1=w16_sb, op=MULT)
                    m2 = btp.tile([128, A], F32, tag="m2", name="m2")
                    nc.vector.tensor_reduce(out=m2, in_=val, axis=X, op=MAX)
                    oh2 = btp.tile([128, A, NACT], F32, tag="oh2", name="oh2", bufs=1)
                    nc.vector.tensor_tensor(
                        out=oh2.rearrange("p a o -> p (a o)"),
                        in0=val.rearrange("p a o -> p (a o)"),
                        in1=bass.AP(tensor=m2.tensor, offset=m2.offset,
                                    ap=[m2.ap[0], [1, A], [0, NACT]]),
                        op=ISGE)
                    qm = btp.tile([128, A, NACT], F32, tag="qm", name="qm", bufs=1)
                    nc.vector.tensor_tensor(
                        out=qm.rearrange("p a o -> p (a o)"),
                        in0=oh2.rearrange("p a o -> p (a o)"),
                        in1=allq_bm[:, bt * 128:(bt + 1) * 128],
                        op=MULT)
                    q_sb = btp.tile([128, A], F32, tag="qs", name="qs")
                    nc.vector.tensor_reduce(out=q_sb, in_=qm, axis=X, op=ADD)
                    nc.sync.dma_start(out=q_d[b0:b0 + 128], in_=q_sb)

    nc.compile()
    return nc


def _ap64(t, eh, col, SC):
    """64 partitions at base eh; 128 contiguous cols at `col`."""
    row = A * SC  # free elements per partition of the [128, SC*A] tile
    return bass.AP(tensor=t.tensor, offset=t.offset + eh * row + col,
                   ap=[[row, 64], [1, 128]])


def _prep_inputs(states, actions, enc_W, enc_b, s_W, s_b, key_W, sel_W,
                 val_W, val_b, c_W1, c_b1, c_W2, c_b2,
                 b_local=B_LOCAL, n_cores=N_CORES):
    f32 = np.float32
    Bv = b_local * n_cores
    states = states[:, :Bv]
    actions = actions[:, :Bv]
    sa = np.concatenate([states, actions], axis=-1).astype(f32)
    saT = np.zeros((A, KPAD, Bv), dtype=bfloat16)
    saT[:, :IDIM, :] = sa.transpose(0, 2, 1).astype(bfloat16)
    SCW = min(512, b_local)
    n_sc = b_local // SCW
    w_enc = np.zeros((A, KPAD, 2 * H), dtype=bfloat16)
    w_enc[:, :IDIM, :H] = enc_W.astype(bfloat16)
    w_enc[:, :S, H:] = s_W.astype(bfloat16)
    b_enc = np.concatenate([enc_b, s_b], axis=-1).astype(f32).reshape(A, 4, 128, 1)
    w_ks = np.zeros((H, 2 * H), dtype=bfloat16)
    w_ks[:, 0:H] = key_W.transpose(1, 0, 2).reshape(H, H).astype(bfloat16)
    w_ks[:, H:] = sel_W.transpose(1, 0, 2).reshape(H, H).astype(bfloat16)
    w_val = val_W.transpose(1, 0, 2).reshape(H, H).astype(bfloat16)
    bv = np.tile(val_b.reshape(1, -1), (1, 2)).astype(bfloat16)  # doubled: 2 agents/psum
    w_c1 = c_W1.astype(bfloat16)
    b_c1 = c_b1.astype(f32).reshape(A, 2, 128, 1)
    w_c2 = c_W2.astype(bfloat16)
    b_c2 = c_b2.astype(f32).reshape(A, NACT, 1)
    # dmask[(j*16+b2), (i*16+b1)] = (b2==b1) & (i != j)
    dmask = np.zeros((128, 128), dtype=bfloat16)
    for b2 in range(16):
        for j in range(A):
            for i in range(A):
                if i != j:
                    dmask[j * 16 + b2, i * 16 + b2] = 1
    w16 = np.broadcast_to(
        (NACT - np.arange(NACT, dtype=f32))[None, None, :],
        (128, A, NACT)).reshape(128, A * NACT).copy()
    acts_bm = actions.transpose(1, 0, 2).astype(f32)

    shared = dict(w_enc=w_enc, b_enc=b_enc, w_ks=w_ks, w_val=w_val, bv=bv,
                  w_c1=w_c1, b_c1=b_c1, w_c2=w_c2, b_c2=b_c2,
                  dmask=dmask, w16=w16)
    in_maps = []
    for cid in range(n_cores):
        sl = slice(cid * b_local, (cid + 1) * b_local)
        m_ = dict(shared)
        # pack [A, KPAD, b_local] -> [A, n_sc, 128, 4, SCW] so each partition's
        # x-load is one contiguous 4KB run
        sc_ = saT[:, :, sl].reshape(A, 4, 128, n_sc, SCW)
        m_["saT"] = np.ascontiguousarray(sc_.transpose(0, 3, 2, 1, 4))
        m_["acts"] = np.ascontiguousarray(acts_bm[sl])
        in_maps.append(m_)
    return in_maps


_NC_CACHE = {}


def _get_nc(b_local=B_LOCAL):
    if b_local not in _NC_CACHE:
        _NC_CACHE[b_local] = build_bass(b_local)
    return _NC_CACHE[b_local]


def kernel(**inputs):
    inputs = {k: np.asarray(v) for k, v in inputs.items()}
    in_maps = _prep_inputs(**inputs)
    nc = _get_nc()
    res = bass_utils.run_bass_kernel_spmd(
        nc, in_maps, core_ids=list(range(N_CORES)),
        trace=bool(int(os.environ.get("MAAC_TRACE", "0"))))
    q = np.concatenate([r["q"].T for r in res.results], axis=1)  # [A, B]
    if res.exec_time_ns is not None:
        print(f"HW exec time: {res.exec_time_ns} ns")
    return q[:, :, None].astype(np.float32)


# revision 6
# speedup vs baseline: 1.0605x; 1.0414x over previous
"""MAAC critic kernel v2 for Trainium2 — attention on the TensorEngine.

Data-parallel over batch on 8 cores. Per core (b_local=2048), processed in
superchunks of SC=512 (b-tiles of 128, chunks of 16):

  encoder/kvs: feature-major weights-stationary matmuls (bf16).
  stage-1 logits: per (chunk16, head) ONE PE matmul
      out[(b2*8+j), (b1*8+i)] = sum_d keys[d,(b2,j)] * sels[d,(b1,i)]
    — the b2==b1 diagonal blocks are the real logits; the rest is cheap waste.
  softmax stays in that domain: exp on the whole block (ScalarE), then one
    DVE multiply with a constant mask [(b2 j),(b1 i)] = (b2==b1)&(i!=j)
    which zeroes both the off-diagonal garbage and the self-agent term.
  stage-2: out[(b1 i), (d|Z)] = Pm^T @ [V2 | ones] — block-diag structure of
    Pm makes this exact; the appended ones-column yields the softmax
    denominator Z per (b, i) for free.  V2 = vals in [(b0*8+j), ed] layout
    via a dense DRAM round trip (the only partition reshuffle).
  normalize: DVE reciprocal of the Z columns + one broadcast multiply.
  transpose attended values back feature-major via PE transposes, then
  per-agent critic MLP, PE-transpose of q, fp32 one-hot gather with a
  first-index tie-break matching jnp.argmax.
"""
import os
import numpy as np
import ml_dtypes

import concourse.bass as bass
import concourse.tile as tile
import concourse.mybir as mybir
from concourse import bacc
from concourse import bass_utils
from concourse.masks import make_identity

F32 = mybir.dt.float32
BF16 = mybir.dt.bfloat16
bfloat16 = ml_dtypes.bfloat16

A = 8
B = 16384
S = 456
NACT = 16
IDIM = S + NACT
KPAD = 512
H = 256
E = 4
D = 64
ED = E * D               # 256
N_CORES = 8
B_LOCAL = B // N_CORES   # 2048
ALPHA = 0.01


def build_bass(b_local=B_LOCAL, lrelu_act=True, stages=5):
    """stages: 1=enc/kvs, 2=+vals/V2, 3=+s1/softmax, 4=+s2/norm/transpose, 5=all."""
    nc = bacc.Bacc("TRN2", target_bir_lowering=False, debug=False)
    SC = min(512, b_local)       # superchunk width (free dim of big matmuls)
    n_sc = b_local // SC
    NBT = SC // 128              # b-tiles per superchunk
    LRELU = mybir.ActivationFunctionType.Lrelu
    EXP = mybir.ActivationFunctionType.Exp
    IDENT = mybir.ActivationFunctionType.Identity
    MULT = mybir.AluOpType.mult
    ADD = mybir.AluOpType.add
    ISGE = mybir.AluOpType.is_ge
    MAX = mybir.AluOpType.max
    X = mybir.AxisListType.X

    # ---- DRAM I/O ----
    saT_d = nc.dram_tensor("saT", [A, b_local // min(512, b_local), 128, 4,
                                   min(512, b_local)], BF16, kind="ExternalInput")
    acts_d = nc.dram_tensor("acts", [b_local, A, NACT], F32, kind="ExternalInput")
    w_enc_d = nc.dram_tensor("w_enc", [A, KPAD, 2 * H], BF16, kind="ExternalInput")
    b_enc_d = nc.dram_tensor("b_enc", [A, 4, 128, 1], F32, kind="ExternalInput")
    w_ks_d = nc.dram_tensor("w_ks", [H, 2 * H], BF16, kind="ExternalInput")
    w_val_d = nc.dram_tensor("w_val", [H, ED], BF16, kind="ExternalInput")
    bv_d = nc.dram_tensor("bv", [1, 2 * ED], BF16, kind="ExternalInput")
    w_c1_d = nc.dram_tensor("w_c1", [A, 2 * H, H], BF16, kind="ExternalInput")
    b_c1_d = nc.dram_tensor("b_c1", [A, 2, 128, 1], F32, kind="ExternalInput")
    w_c2_d = nc.dram_tensor("w_c2", [A, H, NACT], BF16, kind="ExternalInput")
    b_c2_d = nc.dram_tensor("b_c2", [A, NACT, 1], F32, kind="ExternalInput")
    dmask_d = nc.dram_tensor("dmask", [128, 128], BF16, kind="ExternalInput")
    w16_d = nc.dram_tensor("w16", [128, A * NACT], F32, kind="ExternalInput")
    q_d = nc.dram_tensor("q", [b_local, A], F32, kind="ExternalOutput")

    with tile.TileContext(nc) as tc:
        import contextlib
        with contextlib.ExitStack() as ctx:
            wp = ctx.enter_context(tc.tile_pool(name="wp", bufs=1))
            xp = ctx.enter_context(tc.tile_pool(name="xp", bufs=2))
            scp = ctx.enter_context(tc.tile_pool(name="scp", bufs=1))
            btp = ctx.enter_context(tc.tile_pool(name="btp", bufs=2))
            pup = ctx.enter_context(tc.tile_pool(name="pup", bufs=2))
            hp = ctx.enter_context(tc.tile_pool(name="hp", bufs=2))
            pmm = ctx.enter_context(tc.tile_pool(name="pmm", bufs=3, space="PSUM"))
            ps1 = ctx.enter_context(tc.tile_pool(name="ps1", bufs=1, space="PSUM"))
            ps2 = ctx.enter_context(tc.tile_pool(name="ps2", bufs=1, space="PSUM"))
            ptp = ctx.enter_context(tc.tile_pool(name="ptp", bufs=1, space="PSUM"))
            dp = ctx.enter_context(tc.tile_pool(name="dp", bufs=2, space="DRAM"))

            # ---- resident weights ----
            w_enc_sb, b_enc_sb = [], []
            w_c1_sb, b_c1_sb, w_c2_sb, b_c2_sb = [], [], [], []
            for a in range(A):
                w = wp.tile([128, 4, 2 * H], BF16, name=f"w_enc{a}")
                (nc.sync if a == 0 else nc.scalar).dma_start(
                    out=w, in_=w_enc_d[a].rearrange("(kt p) m -> p kt m", p=128))
                w_enc_sb.append(w)
                bt_ = wp.tile([128, 4, 1], F32, name=f"b_enc{a}")
                nc.sync.dma_start(out=bt_, in_=b_enc_d[a].rearrange("kt p one -> p kt one"))
                b_enc_sb.append(bt_)
                w1 = wp.tile([128, 4, H], BF16, name=f"w_c1{a}")
                nc.scalar.dma_start(out=w1, in_=w_c1_d[a].rearrange("(kt p) m -> p kt m", p=128))
                w_c1_sb.append(w1)
                b1 = wp.tile([128, 2, 1], F32, name=f"b_c1{a}")
                nc.scalar.dma_start(out=b1, in_=b_c1_d[a].rearrange("kt p one -> p kt one"))
                b_c1_sb.append(b1)
                w2 = wp.tile([128, 2, NACT], BF16, name=f"w_c2{a}")
                nc.scalar.dma_start(out=w2, in_=w_c2_d[a].rearrange("(kt p) m -> p kt m", p=128))
                w_c2_sb.append(w2)
                b2 = wp.tile([NACT, 1], F32, name=f"b_c2{a}")
                nc.scalar.dma_start(out=b2, in_=b_c2_d[a])
                b_c2_sb.append(b2)
            w_ks_sb = wp.tile([128, 2, 2 * H], BF16, name="w_ks")
            nc.sync.dma_start(out=w_ks_sb, in_=w_ks_d.rearrange("(kt p) m -> p kt m", p=128))
            w_val_sb = wp.tile([128, 2, ED], BF16, name="w_val")
            nc.sync.dma_start(out=w_val_sb, in_=w_val_d.rearrange("(kt p) m -> p kt m", p=128))
            bv2_sb = wp.tile([1, 2 * ED], BF16, name="bv2")
            nc.sync.dma_start(out=bv2_sb, in_=bv_d[:, :])
            ones_sb = wp.tile([1, 128], BF16, name="ones")
            nc.vector.memset(ones_sb, 1.0)
            dmask_sb = wp.tile([128, 128], BF16, name="dmask")
            nc.sync.dma_start(out=dmask_sb, in_=dmask_d[:, :])
            w16_sb = wp.tile([128, A * NACT], F32, name="w16")
            nc.sync.dma_start(out=w16_sb, in_=w16_d[:, :])
            identb = wp.tile([128, 128], BF16, name="identb")
            make_identity(nc, identb)
            ident16 = wp.tile([16, 16], F32, name="ident16")
            make_identity(nc, ident16)

            RELU = mybir.ActivationFunctionType.Relu
            b2_cache = {}

            def evict_lrelu(psum_ap, out_ap, bias, eng=None):
                """out = lrelu(psum + bias); bias is a [128,1] AP or None.

                Table-free on ScalarE (Identity w/ bias) so the Exp act-table
                stays resident; the leak is one max(y, alpha*y) on eng."""
                if lrelu_act:
                    n = psum_ap.shape[-1]
                    tmp = pup.tile([128, SC], BF16, tag="lr", name="lr", bufs=3)
                    tn = tmp[:, :n] if n != SC else tmp
                    nc.scalar.activation(out=tn, in_=psum_ap, func=IDENT,
                                         bias=bias if bias is not None else 0.0,
                                         scale=1.0)
                    (eng or nc.vector).scalar_tensor_tensor(
                        out=out_ap, in0=tn, scalar=ALPHA, op0=MULT,
                        in1=tn, op1=mybir.AluOpType.max)
                    return
                # CoreSim fallback: lrelu(y) = y + relu(-(1-a)*y), y = x + b
                n = psum_ap.shape[-1]
                r = xp.tile([128, SC], F32, tag="lrtmp", name="lrtmp")
                rr = r[:, :n] if n != SC else r
                bias2 = 0.0
                if bias is not None:
                    key = (bias.tensor.name, bias.offset)
                    if key not in b2_cache:
                        b2 = wp.tile([128, 1], F32, name=f"b2_{len(b2_cache)}")
                        nc.vector.tensor_scalar_mul(out=b2, in0=bias,
                                                    scalar1=-(1.0 - ALPHA))
                        b2_cache[key] = b2
                    bias2 = b2_cache[key]
                nc.scalar.activation(out=rr, in_=psum_ap, func=RELU,
                                     bias=bias2, scale=-(1.0 - ALPHA))
                nc.vector.scalar_tensor_tensor(
                    out=out_ap, in0=psum_ap,
                    scalar=bias if bias is not None else 0.0,
                    op0=ADD, in1=rr, op1=ADD)

            for sc in range(n_sc):
                sc0 = sc * SC
                sa_enc = scp.tile([128, A, 2, SC], BF16, tag="sa_enc", name="sa_enc")
                s_enc = scp.tile([128, A, 2, SC], BF16, tag="s_enc", name="s_enc",
                                 bufs=2)
                # keys/sels stored feature-major with cols (chunk, b2, j) so the
                # s1 matmul operands are contiguous 128-col slices
                keys_fm = [scp.tile([128, SC * A], BF16, tag=f"keys{t}", name=f"keys{t}") for t in range(2)]
                sels_fm = [scp.tile([128, SC * A], BF16, tag=f"sels{t}", name=f"sels{t}") for t in range(2)]
                otherT = [scp.tile([128, A * SC], BF16, tag=f"oT{t}", name=f"oT{t}") for t in range(2)]
                allq_bm = scp.tile([128, NBT * 128], F32, tag="allq_bm", name="allq_bm")

                # ---------- encoder + keys/sels (feature-major) ----------
                for a in range(A):
                    x = xp.tile([128, 4, SC], BF16, tag="x", name="x")
                    nc.sync.dma_start(out=x, in_=saT_d[a, sc])
                    for mt in range(4):
                        ps = pmm.tile([128, SC], F32, tag="mm", name="mm")
                        for kt in range(4):
                            nc.tensor.matmul(out=ps,
                                             lhsT=w_enc_sb[a][:, kt, mt * 128:(mt + 1) * 128],
                                             rhs=x[:, kt, :],
                                             start=(kt == 0), stop=(kt == 3))
                        dst = sa_enc[:, a, mt, :] if mt < 2 else s_enc[:, a, mt - 2, :]
                        evict_lrelu(ps, dst, b_enc_sb[a][:, mt, :])
                    # keys (from sa_enc) and sels (from s_enc), feature-major,
                    # evicted into (chunk, j=a, b2) interleaved column order
                    # (16-wide contiguous runs; attention row rank = j*16+b2)
                    def ks_dst(t, a):
                        return bass.AP(tensor=t.tensor, offset=t.offset + a * 16,
                                       ap=[t.ap[0], [128, SC // 16], [1, 16]])
                    for et in range(2):
                        ps = pmm.tile([128, SC], F32, tag="mm", name="mm")
                        for kt in range(2):
                            nc.tensor.matmul(out=ps,
                                             lhsT=w_ks_sb[:, kt, et * 128:(et + 1) * 128],
                                             rhs=sa_enc[:, a, kt, :],
                                             start=(kt == 0), stop=(kt == 1))
                        nc.vector.tensor_copy(out=ks_dst(keys_fm[et], a), in_=ps)
                        ps = pmm.tile([128, SC], F32, tag="mm", name="mm")
                        for kt in range(2):
                            nc.tensor.matmul(out=ps,
                                             lhsT=w_ks_sb[:, kt, H + et * 128:H + (et + 1) * 128],
                                             rhs=s_enc[:, a, kt, :],
                                             start=(kt == 0), stop=(kt == 1))
                        nc.vector.tensor_copy(out=ks_dst(sels_fm[et], a), in_=ps)

                # ---------- per b-tile: vals, V2 bounce, s1, softmax, s2 ----------
                for bt in range(NBT if stages >= 2 else 0):
                    bt0 = bt * 128
                    # vals b-major [128b, (j, ed)]
                    vals_bm = btp.tile([128, A, ED], BF16, tag="vb", name="vb", bufs=1)
                    for a in range(A):
                        ps = pmm.tile([128, SC], F32, tag="mm", name="mm")
                        for kt in range(2):
                            nc.tensor.matmul(out=ps[:, 0:ED],
                                             lhsT=sa_enc[:, a, kt, bt0:bt0 + 128],
                                             rhs=w_val_sb[:, kt, :],
                                             start=(kt == 0), stop=False)
                        nc.tensor.matmul(out=ps[:, 0:ED], lhsT=ones_sb,
                                         rhs=bv2_sb[:, 0:ED], start=False, stop=True)
                        evict_lrelu(ps[:, 0:ED], vals_bm[:, a, :], None)
                    # V2 via DRAM bounce: vdram [c][j][b0][ed]; row rank = j*16+b0
                    vdram = dp.tile([8, A, 16, ED], BF16, tag="vd", name="vd")
                    for j in range(A):
                        nc.sync.dma_start(
                            out=bass.AP(tensor=vdram.tensor, offset=vdram.offset + j * 16 * ED,
                                        ap=[[A * 16 * ED, 8], [ED, 16], [1, ED]]),
                            in_=vals_bm[:, j, :])
                    V2C = 4 * 65  # 260 cols per chunk: 4 x (64 vals + ones col)
                    v2 = btp.tile([128, 8, V2C], BF16, tag="v2", name="v2")
                    for c in range(8):
                        src = bass.AP(tensor=vdram.tensor, offset=vdram.offset + c * A * 16 * ED,
                                      ap=[[ED, 128], [D, 4], [1, D]])
                        dst = bass.AP(tensor=v2.tensor, offset=v2.offset + c * V2C,
                                      ap=[v2.ap[0], [65, 4], [1, D]])
                        nc.gpsimd.dma_start(out=dst, in_=src)
                    # ones columns at e*65+64
                    nc.vector.memset(
                        bass.AP(tensor=v2.tensor, offset=v2.offset + D,
                                ap=[v2.ap[0], [V2C, 8], [65, 4]]), 1.0)

                    if stages < 3:
                        continue
                    othBM = btp.tile([128, 8, 4 * 65], BF16, tag="ob", name="ob",
                                     bufs=1)
                    for c in range(8):
                        # s1: logits c2-block per head.  Base-64 operands
                        # (odd heads) must target a psum tile at bank offset 0,
                        # so e1/e3 get their own tiles.
                        cg = bt * 8 + c  # chunk index within superchunk
                        pev = ps1.tile([128, 256], F32, tag="s1e", name="s1e")
                        pod = [ps1.tile([128, 128], F32, tag=f"s1o{k}", name=f"s1o{k}")
                               for k in range(2)]
                        for e in range(4):
                            et, eh = e // 2, (e % 2) * 64
                            lhsT = _ap64(keys_fm[et], eh, cg * 128, SC)
                            rhs = _ap64(sels_fm[et], eh, cg * 128, SC)
                            out = (pev[:, (e // 2) * 128:(e // 2 + 1) * 128]
                                   if e % 2 == 0 else pod[e // 2])
                            nc.tensor.matmul(out=out, lhsT=lhsT, rhs=rhs,
                                             start=True, stop=True)
                        # exp (scale 1/sqrt(D)); pu cols stay in (e, b1, i) order
                        pu = pup.tile([128, 512], BF16, tag="pu", name="pu")
                        escale = 1.0 / np.sqrt(np.float32(D))
                        nc.scalar.activation(
                            out=bass.AP(tensor=pu.tensor, offset=pu.offset,
                                        ap=[pu.ap[0], [256, 2], [1, 128]]),
                            in_=pev, func=EXP, bias=0.0, scale=escale)
                        for k in range(2):
                            nc.scalar.activation(out=pu[:, (2 * k + 1) * 128:(2 * k + 2) * 128],
                                                 in_=pod[k], func=EXP, bias=0.0,
                                                 scale=escale)
                        # mask: zero off-diagonal blocks and self-agent
                        pm = pup.tile([128, 512], BF16, tag="pm", name="pm")
                        nc.vector.tensor_tensor(
                            out=pm,
                            in0=pu,
                            in1=bass.AP(tensor=dmask_sb.tensor, offset=dmask_sb.offset,
                                        ap=[dmask_sb.ap[0], [0, 4], [1, 128]]),
                            op=MULT)
                        if stages < 4:
                            continue
                        # s2: other_bm + Z column per head
                        p2 = ps2.tile([128, 4 * 65], F32, tag="s2", name="s2")
                        for e in range(4):
                            nc.tensor.matmul(out=p2[:, e * 65:(e + 1) * 65],
                                             lhsT=pm[:, e * 128:(e + 1) * 128],
                                             rhs=v2[:, c, e * 65:(e + 1) * 65],
                                             start=True, stop=True)
                        nc.vector.tensor_copy(out=othBM[:, c, :], in_=p2)

                    if stages < 4:
                        continue
                    # normalize: rZ = 1/Z, othN = othBM * rZ (broadcast over d)
                    rz = btp.tile([128, 8, 4], BF16, tag="rz", name="rz")
                    with nc.allow_low_precision(reason="1/Z in bf16 is within tolerance"):
                        nc.vector.reciprocal(
                            out=rz,
                            in_=bass.AP(tensor=othBM.tensor, offset=othBM.offset + D,
                                        ap=[othBM.ap[0], [4 * 65, 8], [65, 4]]))
                    othN = btp.tile([128, 8 * 4 * D], BF16, tag="on", name="on")
                    # split over chunk halves so the first transpose group can
                    # start before the second half is normalized
                    for ch2 in range(2):
                        nc.vector.tensor_tensor(
                            out=othN[:, ch2 * 4 * 4 * D:(ch2 + 1) * 4 * 4 * D],
                            in0=bass.AP(tensor=othBM.tensor,
                                        offset=othBM.offset + ch2 * 4 * 4 * 65,
                                        ap=[othBM.ap[0], [4 * 65, 4], [65, 4], [1, D]]),
                            in1=bass.AP(tensor=rz.tensor,
                                        offset=rz.offset + ch2 * 16,
                                        ap=[rz.ap[0], [4, 4], [1, 4], [0, D]]),
                            op=MULT)

                    # transpose back to feature-major: otherT[ep][:, (i, bt, c, b1)]
                    # one [128,128] transpose covers both heads of an e-pair
                    for ch2 in range(2):
                        for ep in range(2):
                            pt = ptp.tile([128, 512], BF16, tag="tr", name="tr")
                            for c2 in range(4):
                                cg = ch2 * 4 + c2
                                nc.tensor.transpose(
                                    out=pt[:, c2 * 128:(c2 + 1) * 128],
                                    in_=othN[:, (cg * 4 + ep * 2) * D:(cg * 4 + ep * 2 + 2) * D],
                                    identity=identb)
                            # evict to otherT[ep] cols (i*SC + bt*128 + c*16 + b1);
                            # pt cols decode as (c2, i, b1) with the (i*16+b1) rank
                            dst = bass.AP(
                                tensor=otherT[ep].tensor,
                                offset=otherT[ep].offset + bt0 + ch2 * 64,
                                ap=[otherT[ep].ap[0], [16, 4], [SC, 8], [1, 16]])
                            nc.vector.tensor_copy(out=dst, in_=pt)

                # ---------- critic (per agent over the whole superchunk) ----------
                if stages < 5:
                    nc.vector.memset(allq_bm, 0.0)
                for a in range(A if stages >= 5 else 0):
                    h_t = hp.tile([128, 2, SC], BF16, tag="h", name="h")
                    for mt in range(2):
                        ps = pmm.tile([128, SC], F32, tag="mm", name="mm")
                        for kt in range(2):
                            nc.tensor.matmul(out=ps,
                                             lhsT=w_c1_sb[a][:, kt, mt * 128:(mt + 1) * 128],
                                             rhs=s_enc[:, a, kt, :],
                                             start=(kt == 0), stop=False)
                        for kt in range(2):
                            nc.tensor.matmul(out=ps,
                                             lhsT=w_c1_sb[a][:, 2 + kt, mt * 128:(mt + 1) * 128],
                                             rhs=otherT[kt][:, a * SC:(a + 1) * SC],
                                             start=False, stop=(kt == 1))
                        evict_lrelu(ps, h_t[:, mt, :], b_c1_sb[a][:, mt, :])
                    allq_a = hp.tile([16, SC], F32, tag="aq", name="aq", bufs=1)
                    psq = pmm.tile([128, SC], F32, tag="mm", name="mm")
                    for kt in range(2):
                        nc.tensor.matmul(out=psq[0:NACT, :], lhsT=w_c2_sb[a][:, kt, :],
                                         rhs=h_t[:, kt, :],
                                         start=(kt == 0), stop=(kt == 1))
                    nc.scalar.activation(out=allq_a, in_=psq[0:NACT, :], func=IDENT,
                                         bias=b_c2_sb[a], scale=1.0)
                    ptq = pmm.tile([128, SC], F32, tag="mm", name="mm")
                    for bt in range(NBT):
                        nc.tensor.transpose(out=ptq[:, bt * 16:(bt + 1) * 16],
                                            in_=allq_a[:, bt * 128:(bt + 1) * 128],
                                            identity=ident16)
                    # allq_bm cols (bt, a, o)
                    nc.vector.tensor_copy(
                        out=bass.AP(tensor=allq_bm.tensor,
                                    offset=allq_bm.offset + a * NACT,
                                    ap=[allq_bm.ap[0], [128, NBT], [1, NACT]]),
                        in_=ptq[:, 0:NBT * 16])

                # ---------- argmax gather (per b-tile) ----------
                for bt in range(NBT):
                    b0 = sc0 + bt * 128
                    acts_t = btp.tile([128, A, NACT], F32, tag="at", name="at")
                    nc.sync.dma_start(out=acts_t, in_=acts_d[b0:b0 + 128])
                    amax = btp.tile([128, A], F32, tag="am", name="am")
                    nc.vector.tensor_reduce(out=amax, in_=acts_t, axis=X, op=MAX)
                    oh1 = btp.tile([128, A, NACT], F32, tag="oh1", name="oh1", bufs=1)
                    nc.vector.tensor_tensor(
                        out=oh1.rearrange("p a o -> p (a o)"),
                        in0=acts_t.rearrange("p a o -> p (a o)"),
                        in1=bass.AP(tensor=amax.tensor, offset=amax.offset,
                                    ap=[amax.ap[0], [1, A], [0, NACT]]),
                        op=ISGE)
                    # tie-break: keep only the first (lowest-index) max
                    val = btp.tile([128, A, NACT], F32, tag="val", name="val", bufs=1)
                    nc.vector.tensor_tensor(out=val.rearrange("p a o -> p (a o)"),
                                            in0=oh1.rearrange("p a o -> p (a o)"),
                                            in1=w16_sb, op=MULT)
                    m2 = btp.tile([128, A], F32, tag="m2", name="m2")
                    nc.vector.tensor_reduce(out=m2, in_=val, axis=X, op=MAX)
                    oh2 = btp.tile([128, A, NACT], F32, tag="oh2", name="oh2", bufs=1)
                    nc.vector.tensor_tensor(
                        out=oh2.rearrange("p a o -> p (a o)"),
                        in0=val.rearrange("p a o -> p (a o)"),
                        in1=bass.AP(tensor=m2.tensor, offset=m2.offset,
                                    ap=[m2.ap[0], [1, A], [0, NACT]]),
                        op=ISGE)
                    qm = btp.tile([128, A, NACT], F32, tag="qm", name="qm", bufs=1)
                    nc.vector.tensor_tensor(
                        out=qm.rearrange("p a o -> p (a o)"),
                        in0=oh2.rearrange("p a o -> p (a o)"),
                        in1=allq_bm[:, bt * 128:(bt + 1) * 128],
                        op=MULT)
                    q_sb = btp.tile([128, A], F32, tag="qs", name="qs")
                    nc.vector.tensor_reduce(out=q_sb, in_=qm, axis=X, op=ADD)
                    nc.sync.dma_start(out=q_d[b0:b0 + 128], in_=q_sb)

    nc.compile()
    return nc


def _ap64(t, eh, col, SC):
    """64 partitions at base eh; 128 contiguous cols at `col`."""
    row = A * SC  # free elements per partition of the [128, SC*A] tile
    return bass.AP(tensor=t.tensor, offset=t.offset + eh * row + col,
                   ap=[[row, 64], [1, 128]])


def _prep_inputs(states, actions, enc_W, enc_b, s_W, s_b, key_W, sel_W,
                 val_W, val_b, c_W1, c_b1, c_W2, c_b2,
                 b_local=B_LOCAL, n_cores=N_CORES):
    f32 = np.float32
    Bv = b_local * n_cores
    states = states[:, :Bv]
    actions = actions[:, :Bv]
    sa = np.concatenate([states, actions], axis=-1).astype(f32)
    saT = np.zeros((A, KPAD, Bv), dtype=bfloat16)
    saT[:, :IDIM, :] = sa.transpose(0, 2, 1).astype(bfloat16)
    SCW = min(512, b_local)
    n_sc = b_local // SCW
    w_enc = np.zeros((A, KPAD, 2 * H), dtype=bfloat16)
    w_enc[:, :IDIM, :H] = enc_W.astype(bfloat16)
    w_enc[:, :S, H:] = s_W.astype(bfloat16)
    b_enc = np.concatenate([enc_b, s_b], axis=-1).astype(f32).reshape(A, 4, 128, 1)
    w_ks = np.zeros((H, 2 * H), dtype=bfloat16)
    w_ks[:, 0:H] = key_W.transpose(1, 0, 2).reshape(H, H).astype(bfloat16)
    w_ks[:, H:] = sel_W.transpose(1, 0, 2).reshape(H, H).astype(bfloat16)
    w_val = val_W.transpose(1, 0, 2).reshape(H, H).astype(bfloat16)
    bv = np.tile(val_b.reshape(1, -1), (1, 2)).astype(bfloat16)  # doubled: 2 agents/psum
    w_c1 = c_W1.astype(bfloat16)
    b_c1 = c_b1.astype(f32).reshape(A, 2, 128, 1)
    w_c2 = c_W2.astype(bfloat16)
    b_c2 = c_b2.astype(f32).reshape(A, NACT, 1)
    # dmask[(j*16+b2), (i*16+b1)] = (b2==b1) & (i != j)
    dmask = np.zeros((128, 128), dtype=bfloat16)
    for b2 in range(16):
        for j in range(A):
            for i in range(A):
                if i != j:
                    dmask[j * 16 + b2, i * 16 + b2] = 1
    w16 = np.broadcast_to(
        (NACT - np.arange(NACT, dtype=f32))[None, None, :],
        (128, A, NACT)).reshape(128, A * NACT).copy()
    acts_bm = actions.transpose(1, 0, 2).astype(f32)

    shared = dict(w_enc=w_enc, b_enc=b_enc, w_ks=w_ks, w_val=w_val, bv=bv,
                  w_c1=w_c1, b_c1=b_c1, w_c2=w_c2, b_c2=b_c2,
                  dmask=dmask, w16=w16)
    in_maps = []
    for cid in range(n_cores):
        sl = slice(cid * b_local, (cid + 1) * b_local)
        m_ = dict(shared)
        # pack [A, KPAD, b_local] -> [A, n_sc, 128, 4, SCW] so each partition's
        # x-load is one contiguous 4KB run
        sc_ = saT[:, :, sl].reshape(A, 4, 128, n_sc, SCW)
        m_["saT"] = np.ascontiguousarray(sc_.transpose(0, 3, 2, 1, 4))
        m_["acts"] = np.ascontiguousarray(acts_bm[sl])
        in_maps.append(m_)
    return in_maps


_NC_CACHE = {}


def _get_nc(b_local=B_LOCAL):
    if b_local not in _NC_CACHE:
        _NC_CACHE[b_local] = build_bass(b_local)
    return _NC_CACHE[b_local]


def kernel(**inputs):
    inputs = {k: np.asarray(v) for k, v in inputs.items()}
    in_maps = _prep_inputs(**inputs)
    nc = _get_nc()
    res = bass_utils.run_bass_kernel_spmd(
        nc, in_maps, core_ids=list(range(N_CORES)),
        trace=bool(int(os.environ.get("MAAC_TRACE", "0"))))
    q = np.concatenate([r["q"].T for r in res.results], axis=1)  # [A, B]
    if res.exec_time_ns is not None:
        print(f"HW exec time: {res.exec_time_ns} ns")
    return q[:, :, None].astype(np.float32)
